# revision 1
# baseline (speedup 1.0000x reference)
"""Multi-head causal attention on 8 Trainium2 NeuronCores.

Problem (hardcoded): batch_x [4, 2048, 1024], 16 heads x 64 head_size,
stacked per-head QKV params, causal softmax attention, output projection.

Sharding: 8 cores = 4 batches x 2 head-groups (8 heads each).  Each core
computes, for its (batch, head-group):
    QT/KT [hd=512, T] and V [T, hd=512] projections,
    ST = K @ Q^T per head (scores transposed: s on partitions, t free),
    P = exp(ST/8) with causal masking (upper s-blocks skipped entirely,
        diagonal blocks multiplied post-exp by a host-provided 0/1 mask),
    OT = V'^T @ P accumulated over s-blocks, where V' = [V | ones] so the
        softmax denominator accumulates in PSUM row 64 (M=65 matmuls),
    OT_norm = OT * (1/den)  (VectorE reciprocal + GPSIMD partition bcast),
    y_partial = OT_norm^T @ Wp_rows  (row-sharded output projection).
Host sums the two partials per batch and adds bp.

Dtypes: projections run in float32r (full-rate fp32 on the PE at N>=256);
the attention probabilities/V (pt, vp), masks, and the whole output path
(otn, Wp, y) are bf16 — same PE rate, half the SBUF/DMA, ~1.6e-3 rel err
vs the 2e-2 gate.  PSUM accumulation is always f32.

Schedule: software-pipelined for PE occupancy (TimelineSim ~254us/core vs
~317us for the naive phase-sequential schedule; PE busy ~236us):
  - exp() on the scalar engine is the per-block critical resource during
    attention, so projection matmuls for chunk c+1 and delayed output
    projections are emitted as "filler" PE work between attention blocks
    (outproj(0) fills window 1; outproj(1)+(2) fill the Act-heaviest
    window 3; remainder drains at window end).
  - PV matmuls lag scores by PV_LAG blocks so exp/mask latency and the
    previous pair's normalize chain stay off the PE critical path; each
    pair runs its masked diagonal blocks first.
  - fillers are drained ahead of each pair's normalize so PSUM-recycling
    moves aren't stuck behind it on the in-order DVE queue.
  - the final output projection runs 6 PSUM banks wide (reusing the done
    ot/st pool banks), splits its last-pair matmuls K=64 to read the
    un-shifted head-b tile, and alternates result copies Act/DVE.
  - first-chunk input DMAs (split small) precede the one-time weight
    loads so the PE starts ~5us in instead of ~26us.
"""

import numpy as np
from collections import deque
from contextlib import ExitStack

import concourse.bass as bass
import concourse.bacc as bacc
import concourse.mybir as mybir
import concourse.tile as tile
from concourse import library_config
from concourse.bass_utils import run_bass_kernel_spmd

# problem shape (hardcoded per contest rules)
B = 4
T = 2048
E = 1024
NH = 16          # total heads
D = 64           # head size
SCALE = 1.0 / 8.0  # 1/sqrt(64)

# per-core decomposition
H = 8            # heads per core
NPAIR = 4        # head pairs per core
TCH = 512        # t-chunk (matmul free dim)
NTCH = T // TCH  # 4
P = 128
ECH = E // P     # 8 e-chunks
NSB = T // P     # 16 s-blocks
N_CORES = 8
PV_LAG = 8       # blocks between score emission and PV emission

F32 = mybir.dt.float32
F32R = mybir.dt.float32r
BF16 = mybir.dt.bfloat16
AF = mybir.ActivationFunctionType
ALU = mybir.AluOpType


def _emit(nc, tc, io):
    xT, wq, wk, wv, bq, bk, bvb, wp, msk, y = (
        io["xT"], io["wq"], io["wk"], io["wv"], io["bq"], io["bk"],
        io["bvb"], io["wp"], io["msk"], io["y"])

    ctx = ExitStack()
    with ctx:
        # ---- resident SBUF pools (bufs=1) ----
        res = ctx.enter_context(tc.tile_pool(name="res", bufs=1))
        kt_all = res.tile([P, NPAIR, T], F32R)          # KT: partitions = pair-hd
        vp_all = res.tile([P, NSB, H, D + 1], BF16)     # V' = [V | ones]
        wv_sb = res.tile([P, ECH, TCH], F32R)
        wp_sb = res.tile([P, NPAIR, 2, TCH], BF16)
        bq_sb = res.tile([P, NPAIR], F32)
        bk_sb = res.tile([P, NPAIR], F32)
        bv_sb = res.tile([P, TCH], F32)
        msk_sb = res.tile([P, TCH], BF16)
        # pair-3 head-b rows of Wp duplicated at partitions 0-63 for the
        # final outproj's K=64 split (otb lives on partitions 0-63)
        wp_hib = res.tile([D, 2, TCH], BF16)

        # ---- cycling pools ----
        xe_pool = ctx.enter_context(tc.tile_pool(name="xe", bufs=8))
        wqk_pool = ctx.enter_context(tc.tile_pool(name="wqk", bufs=8))
        qt_pool = ctx.enter_context(tc.tile_pool(name="qt", bufs=2))
        pt_pool = ctx.enter_context(tc.tile_pool(name="pt", bufs=10))
        otn_pool = ctx.enter_context(tc.tile_pool(name="otn", bufs=12))
        rden_pool = ctx.enter_context(tc.tile_pool(name="rden", bufs=2))
        bc_pool = ctx.enter_context(tc.tile_pool(name="bc", bufs=1))
        otb_pool = ctx.enter_context(tc.tile_pool(name="otb", bufs=1))
        ysb_pool = ctx.enter_context(tc.tile_pool(name="ysb", bufs=4))
        ps512 = ctx.enter_context(tc.tile_pool(name="ps512", bufs=2, space="PSUM"))
        st_pool = ctx.enter_context(tc.tile_pool(name="stp", bufs=2, space="PSUM"))
        ot_pool = ctx.enter_context(tc.tile_pool(name="otp", bufs=2, space="PSUM"))

        dma = nc.sync.dma_start
        nc.gpsimd.load_library(library_config.attnmlp)

        xeg = {}   # c -> {gi: xe quarter tile}
        qt_c = {}  # c -> qt tile [P, NPAIR, TCH]
        otn_all = {}  # (c, p) -> otn tile [P, TCH]
        otb_last = [None]  # un-shifted head-b tile of the very last pair
        proj_rest_loads = {}  # c -> fire the deferred wqk loads
        proj_loaders = {}  # c -> load_wt(n) for fine-grained prologue order

        def emit_xe_dma(c, quarters=(0, 1, 2, 3)):
            # quarter tiles (2 e-chunks each) so the prologue's first matmul
            # only waits on a 0.5 MB DMA instead of the full 2 MB
            t0 = c * TCH
            xv = xT[:, t0:t0 + TCH].rearrange("(g e r) t -> g r e t", g=4, r=P)
            for gi in quarters:
                xt = xe_pool.tile([P, 2, TCH], F32R, name=f"xe{c}_{gi}", tag="xe")
                if c == 0 and gi == 0:
                    # split so the kernel's first matmul (needs only e=0)
                    # starts after a 0.25 MB transfer
                    dma(xt[:, 0, :], xv[gi][:, 0, :])
                    dma(xt[:, 1, :], xv[gi][:, 1, :])
                else:
                    dma(xt, xv[gi])
                xeg.setdefault(c, {})[gi] = xt

        def proj_closures(c, preload=8):
            """Closure list for chunk c's QKV projections, in emission order
            [q(p0), k(p0), V(i0..3), q(p1), k(p1), ..., q(p3), k(p3)] — the
            prologue drains the first 6 chains dense, window fillers take the
            rest.  Each chain is 8 matmuls + a DVE move w/ bias.  The first
            `preload` wqk weight DMAs fire at build time (a full window ahead
            of use); the rest via the stashed loader in proj_rest_loads."""
            t0 = c * TCH

            def xe_rhs(e):
                return xeg[c][e // 2][:, e % 2, :]

            def xe_lhs(e, i):
                return xeg[c][e // 2][:, e % 2, i * P:(i + 1) * P]

            qt_c[c] = qt_pool.tile([P, NPAIR, TCH], F32R, name=f"qtc{c}",
                                   tag="qt")
            chains = []  # (wdram, b_sb, dest, label) per q/k chain
            for p in range(NPAIR):
                chains.append((wq, bq_sb, p, qt_c[c][:, p, :], f"q{c}_{p}"))
                chains.append((wk, bk_sb, p, kt_all[:, p, t0:t0 + TCH],
                               f"k{c}_{p}"))
            wts = [None] * len(chains)

            def load_wt(n):
                wdram, _, p, _, label = chains[n]
                wt = wqk_pool.tile([P, ECH, P], F32R, name=f"w{label}",
                                   tag="wqk")
                src = wdram[:, p * P:(p + 1) * P].rearrange("(e r) m -> r e m",
                                                            r=P)
                if c == 0 and n == 0:
                    # halved for the same reason as the first xe quarter
                    dma(wt[:, 0:4, :], src[:, 0:4, :])
                    dma(wt[:, 4:8, :], src[:, 4:8, :])
                else:
                    dma(wt, src)
                wts[n] = wt

            for n in range(preload):
                load_wt(n)
            proj_rest_loads[c] = lambda: [load_wt(n)
                                          for n in range(preload, len(chains))]
            proj_loaders[c] = load_wt  # per-index firing for the prologue

            def qk_ops(n):
                ops = []
                _, b_sb, p, dest, label = chains[n]
                ps = ps512.tile([P, TCH], F32, name=f"ps{label}", tag="ps512")
                for e in range(ECH):
                    def mm(n=n, ps=ps, e=e):
                        nc.tensor.matmul(ps[:], wts[n][:, e, :], xe_rhs(e),
                                         start=(e == 0), stop=(e == ECH - 1),
                                         skip_group_check=True)
                    ops.append(mm)

                def mv(b_sb=b_sb, p=p, dest=dest, ps=ps):
                    # on Act, not DVE: at window boundaries the next window's
                    # first scores wait on this move, and the DVE queue is
                    # clogged with the last pair's normalize; Act is idle
                    # there (exps done) and has slack mid-window 0-2
                    nc.scalar.activation(dest, ps[:], AF.Identity,
                                         bias=b_sb[:, p:p + 1])
                ops.append(mv)
                return ops

            def v_ops(i):
                ops = []
                tt = 4 * c + i
                ps = ps512.tile([P, TCH], F32, name=f"v{c}_{i}", tag="ps512")
                for e in range(ECH):
                    def mm(ps=ps, e=e, i=i):
                        nc.tensor.matmul(ps[:], xe_lhs(e, i), wv_sb[:, e, :],
                                         start=(e == 0), stop=(e == ECH - 1),
                                         skip_group_check=True)
                    ops.append(mm)

                def mv(ps=ps, tt=tt):
                    nc.vector.tensor_add(
                        vp_all[:, tt, :, 0:D],
                        ps[:].rearrange("p (h d) -> p h d", d=D),
                        bv_sb[:].rearrange("p (h d) -> p h d", d=D))
                ops.append(mv)
                return ops

            ops = qk_ops(0) + qk_ops(1)
            for i in range(4):
                ops += v_ops(i)
            for n in range(2, len(chains)):
                ops += qk_ops(n)
            return ops

        def outproj_closures(cc):
            """Filler closures for the output projection of chunk cc."""
            ops = []
            for i in range(4):
                tt = 4 * cc + i
                ysb = ysb_pool.tile([P, 2, TCH], BF16, name=f"ysb{cc}_{i}",
                                    tag="ysb")
                for ec in range(2):
                    yp = ps512.tile([P, TCH], F32, name=f"y{cc}_{i}_{ec}",
                                    tag="ps512")
                    for p in range(NPAIR):
                        def mm(cc=cc, i=i, ec=ec, p=p, yp=yp):
                            nc.tensor.matmul(
                                yp[:], otn_all[(cc, p)][:, i * P:(i + 1) * P],
                                wp_sb[:, p, ec, :],
                                start=(p == 0), stop=(p == NPAIR - 1),
                                skip_group_check=True)
                        ops.append(mm)

                    def cp(ysb=ysb, ec=ec, yp=yp):
                        nc.vector.tensor_copy(ysb[:, ec, :], yp[:])
                    ops.append(cp)

                def st(tt=tt, ysb=ysb):
                    dma(y[tt * P:(tt + 1) * P, :],
                        ysb[:].rearrange("p a b -> p (a b)"))
                ops.append(st)
            return ops

        def normalize(c, p, ot_a, ot_b):
            rden_a = rden_pool.tile([1, TCH], F32, name=f"rda{c}_{p}", tag="rd")
            rden_b = rden_pool.tile([1, TCH], F32, name=f"rdb{c}_{p}", tag="rd")
            nc.vector.reciprocal(rden_a[:], ot_a[D:D + 1, :])
            nc.vector.reciprocal(rden_b[:], ot_b[D:D + 1, :])
            bc_a = bc_pool.tile([D, TCH], F32, name=f"bca{c}_{p}", tag="bca")
            bc_b = bc_pool.tile([D, TCH], F32, name=f"bcb{c}_{p}", tag="bcb")
            nc.gpsimd.partition_broadcast(bc_a[:], rden_a[:], channels=D)
            nc.gpsimd.partition_broadcast(bc_b[:], rden_b[:], channels=D)
            otn = otn_pool.tile([P, TCH], BF16, name=f"otn{c}_{p}", tag="otn")
            otb = otb_pool.tile([D, TCH], BF16, name=f"otb{c}_{p}", tag="otb")
            nc.vector.tensor_mul(otn[0:D, :], ot_a[0:D, :], bc_a[:])
            nc.vector.tensor_mul(otb[:], ot_b[0:D, :], bc_b[:])
            if (c, p) == (NTCH - 1, NPAIR - 1):
                # last pair of the kernel: the final outproj reads otb
                # directly via a K=64 matmul split, so the ~2us DMA shift
                # latency stays off the critical tail
                otb_last[0] = otb
            else:
                # partition shift 0:64 -> 64:128 (DMA; DVE lanes can't shift)
                dma(otn[D:2 * D, :], otb[:])
            otn_all[(c, p)] = otn

        def window(c, fillers, guards=None):
            """Attention for chunk c, interleaved with filler PE work.

            guards[p] = minimum number of fillers that must be EMITTED before
            pair p's first score (Tile versioning is emission-ordered, so a
            score reading qt/kt written by a not-yet-emitted filler move would
            silently read the previous version)."""
            nb = 4 * (c + 1)  # causal s-blocks for this chunk
            popped = [0]

            def pop_filler():
                if fillers:
                    fillers.popleft()()
                    popped[0] += 1

            for p in range(NPAIR):
                if guards:
                    while popped[0] < guards.get(p, 0) and fillers:
                        pop_filler()
                ot_a = ot_pool.tile([D + 1, TCH], F32, name=f"ota{c}_{p}",
                                    tag="ot")
                ot_b = ot_pool.tile([D + 1, TCH], F32, name=f"otb{c}_{p}",
                                    tag="ot")
                pv_q = deque()
                # diagonal blocks first: their mask multiplies land on the
                # in-order DVE queue while it still has the previous pair's
                # normalize queued, and the PV lag covers both; the unmasked
                # wide blocks then flush the pair densely.  Pair 0 keeps them
                # last — at window start DVE is busy with boundary moves.
                ks = list(range(4 * c, nb)) + list(range(0, 4 * c))
                for ki, k in enumerate(ks):
                    # diagonal blocks: columns < 128j are fully masked; trim
                    # all work to the live range [t1:TCH] (t1 = 128j).
                    j = k - 4 * c
                    t1 = 128 * j if j > 0 else 0
                    w = TCH - t1
                    st = st_pool.tile([P, 2, TCH], F32, name=f"st{c}_{p}_{k}",
                                      tag="st")
                    pt = pt_pool.tile([P, 2, TCH], BF16, name=f"pt{c}_{p}_{k}",
                                      tag="pt")
                    for h in (0, 1):
                        lo = 64 * h
                        nc.tensor.matmul(
                            st[:, h, t1:TCH],
                            kt_all[lo:lo + 64, p, k * P:(k + 1) * P],
                            qt_c[c][lo:lo + 64, p, t1:TCH],
                            start=True, stop=True)
                    nc.scalar.activation(pt[:, :, t1:TCH], st[:, :, t1:TCH],
                                         AF.Exp, scale=SCALE)
                    if j >= 0:
                        # the mask is only != 1 inside the 128-wide diagonal
                        # square [t1, t1+128): beyond it t >= 128j+s for all
                        # s, so the multiply needn't touch those columns
                        nc.vector.tensor_mul(
                            pt[:, :, t1:t1 + P], pt[:, :, t1:t1 + P],
                            msk_sb[:, 0:P].rearrange("p (a w) -> p a w", a=1)
                            .broadcast_to([P, 2, P]))

                    def pv(k=k, ki=ki, t1=t1, pt=pt, ot_a=ot_a, ot_b=ot_b):
                        st_flag = (ki == 0)
                        sp_flag = (ki == nb - 1)
                        nc.tensor.matmul(ot_a[:, t1:TCH],
                                         vp_all[:, k, 2 * p, :],
                                         pt[:, 0, t1:TCH],
                                         start=st_flag, stop=sp_flag,
                                         skip_group_check=True)
                        nc.tensor.matmul(ot_b[:, t1:TCH],
                                         vp_all[:, k, 2 * p + 1, :],
                                         pt[:, 1, t1:TCH],
                                         start=st_flag, stop=sp_flag,
                                         skip_group_check=True)
                    pv_q.append(pv)
                    if len(pv_q) > PV_LAG:
                        pv_q.popleft()()
                    pop_filler()
                while pv_q:
                    pv_q.popleft()()
                # let filler moves/copies go ahead of this pair's normalize on
                # the in-order DVE queue — they recycle PSUM banks that pending
                # PE matmuls wait on (drain everything before the last pair)
                for _ in range(6 if p < NPAIR - 1 else len(fillers)):
                    pop_filler()
                normalize(c, p, ot_a, ot_b)
            # drain remaining fillers before the next chunk's attention
            while fillers:
                fillers.popleft()()

        # ================= prologue =================
        # First-chunk inputs first so the PE can start ASAP; one-time loads
        # are ordered by first use (wv quartered for the dense V chains,
        # masks/biases next, wp deferred into window 0's fillers).
        # DMA order puts the first matmul's two dependencies (xe e0-slice +
        # first weight half) adjacent at the queue head, then interleaves
        # the rest in consumption order.
        xv0 = xT[:, 0:TCH].rearrange("(g e r) t -> g r e t", g=4, r=P)
        xt0 = xe_pool.tile([P, 2, TCH], F32R, name="xe0_0", tag="xe")
        xeg[0] = {0: xt0}
        dma(xt0[:, 0, :], xv0[0][:, 0, :])           # x e-chunk 0
        fill0 = deque(proj_closures(0, preload=0))
        proj_loaders[0](0)                           # w(q0), halved
        dma(xt0[:, 1, :], xv0[0][:, 1, :])           # x e-chunk 1
        emit_xe_dma(0, quarters=(1,))                # e2,3
        proj_loaders[0](1)                           # w(k0)
        emit_xe_dma(0, quarters=(2, 3))              # e4..7
        dma(bq_sb[:], bq.rearrange("(p r) o -> r (p o)", r=P))
        dma(bk_sb[:], bk.rearrange("(p r) o -> r (p o)", r=P))
        dma(bv_sb[:], bvb[:, :])
        wv_view = wv.rearrange("(e r) n -> r e n", r=P)
        for g in range(4):
            dma(wv_sb[:, 2 * g:2 * g + 2, :], wv_view[:, 2 * g:2 * g + 2, :])
        dma(msk_sb[:], msk[:, :])
        nc.vector.memset(vp_all[:, :, :, D:D + 1], 1.0)
        for n in range(2, 8):
            proj_loaders[0](n)
        for _ in range(6 * 9):  # q(p0), k(p0), V(i0..3) dense
            fill0.popleft()()

        def load_wp():
            for ec in range(2):
                dma(wp_sb[:, :, ec, :],
                    wp[:, ec * TCH:(ec + 1) * TCH]
                    .rearrange("(p r) n -> r p n", r=P))
                dma(wp_hib[:, ec, :],
                    wp[(NPAIR - 1) * P + D:NPAIR * P,
                       ec * TCH:(ec + 1) * TCH])

        # ================= main windows =================
        # outproj(0) fills window 1; outproj(1)+(2) fill window 3, which has
        # no projection work left but the most attention (Act-bound) blocks
        op_sched = {1: (0,), 2: (), 3: (1, 2)}
        for c in range(NTCH):
            fillers = deque()
            guards = None
            if c == 0:
                fillers.extend(fill0)  # q/k chains for pairs 1..3
                fillers.append(load_wp)
                guards = {1: 18, 2: 36, 3: 54}
            for cc in op_sched.get(c, ()):
                fillers.extend(outproj_closures(cc))
            if c + 1 < NTCH:
                emit_xe_dma(c + 1)
                fillers.extend(proj_closures(c + 1))
            window(c, fillers, guards)

        # ============ final output projection (chunk 3) ============
        # Emitted so PE never head-of-line blocks on the last pair's
        # normalize: p0-p2 accumulations (ready) go first two chains at a
        # time; each chain's p3 is split into two K=64 matmuls (otn head-a +
        # un-shifted otb head-b); result copies ride the now-idle Act engine.
        cc = NTCH - 1
        chains = [(i, ec) for i in range(4) for ec in range(2)]
        yps = {}
        ysbs = {}
        # attention is over: the ot/st pools' PSUM banks are free and their
        # slot sizes fit a [P, TCH] f32 accumulator, so the final outproj can
        # run 6 chains in flight instead of 2
        # ot slots last: their previous occupants (the final pair's ot_a/b)
        # are only freed by the last normalize's reads
        yf_pools = [(ps512, "ps512"), (ps512, "ps512"), (st_pool, "st"),
                    (st_pool, "st"), (ot_pool, "ot"), (ot_pool, "ot")]

        def p012(n):
            i, ec = chains[n]
            pool, tag = yf_pools[n % len(yf_pools)]
            yp = pool.tile([P, TCH], F32, name=f"yf{i}_{ec}", tag=tag)
            yps[n] = yp
            for p in range(3):
                nc.tensor.matmul(yp[:], otn_all[(cc, p)][:, i * P:(i + 1) * P],
                                 wp_sb[:, p, ec, :], start=(p == 0),
                                 stop=False, skip_group_check=True)

        def p3_and_copy(n):
            i, ec = chains[n]
            yp = yps[n]
            otn3 = otn_all[(cc, NPAIR - 1)]
            otb3 = otb_last[0]
            nc.tensor.matmul(yp[:], otn3[0:D, i * P:(i + 1) * P],
                             wp_sb[0:D, NPAIR - 1, ec, :],
                             start=False, stop=False, skip_group_check=True)
            nc.tensor.matmul(yp[:], otb3[:, i * P:(i + 1) * P],
                             wp_hib[:, ec, :],
                             start=False, stop=True, skip_group_check=True)
            if i not in ysbs:
                ysbs[i] = ysb_pool.tile([P, 2, TCH], BF16, name=f"ysbf{i}",
                                        tag="ysb")
            if n % 2 == 0:
                nc.scalar.activation(ysbs[i][:, ec, :], yp[:], AF.Copy)
            else:
                nc.vector.tensor_copy(ysbs[i][:, ec, :], yp[:])
            if ec == 1:
                tt = 4 * cc + i
                dma(y[tt * P:(tt + 1) * P, :],
                    ysbs[i][:].rearrange("p a b -> p (a b)"))

        for n in range(6):
            p012(n)
        for n in range(len(chains)):
            p3_and_copy(n)
            if n + 6 < len(chains):
                p012(n + 6)


def build():
    nc = bacc.Bacc(trn_type="TRN2", target_bir_lowering=False, debug=False)
    io = {
        "xT": nc.dram_tensor("xT", [E, T], F32R, kind="ExternalInput").ap(),
        "wq": nc.dram_tensor("wq", [E, H * D], F32R, kind="ExternalInput").ap(),
        "wk": nc.dram_tensor("wk", [E, H * D], F32R, kind="ExternalInput").ap(),
        "wv": nc.dram_tensor("wv", [E, H * D], F32R, kind="ExternalInput").ap(),
        "bq": nc.dram_tensor("bq", [H * D, 1], F32, kind="ExternalInput").ap(),
        "bk": nc.dram_tensor("bk", [H * D, 1], F32, kind="ExternalInput").ap(),
        "bvb": nc.dram_tensor("bvb", [P, H * D], F32, kind="ExternalInput").ap(),
        "wp": nc.dram_tensor("wp", [H * D, E], BF16, kind="ExternalInput").ap(),
        "msk": nc.dram_tensor("msk", [P, TCH], BF16, kind="ExternalInput").ap(),
        "y": nc.dram_tensor("y", [T, E], BF16, kind="ExternalOutput").ap(),
    }
    with tile.TileContext(nc) as tc:
        _emit(nc, tc, io)
    nc.compile()
    return nc


def shard_inputs(inputs):
    """Full inputs -> per-core in_maps (8 cores: batch-major, group-minor)."""
    bx = np.asarray(inputs["batch_x"], np.float32)
    Wq = np.asarray(inputs["Wq"], np.float32)
    Wk = np.asarray(inputs["Wk"], np.float32)
    Wv = np.asarray(inputs["Wv"], np.float32)
    bq = np.asarray(inputs["bq"], np.float32)
    bk = np.asarray(inputs["bk"], np.float32)
    bv = np.asarray(inputs["bv"], np.float32)
    Wp = np.asarray(inputs["Wp"], np.float32)

    import ml_dtypes
    ps = np.arange(P, dtype=np.float32)[:, None]
    tf = np.arange(TCH, dtype=np.float32)[None, :]
    msk = (tf >= ps).astype(ml_dtypes.bfloat16)

    in_maps = []
    for core in range(N_CORES):
        b, g = core // 2, core % 2
        hs = slice(g * H, (g + 1) * H)
        in_maps.append({
            "xT": np.ascontiguousarray(bx[b].T),
            "wq": np.ascontiguousarray(Wq[hs].transpose(1, 0, 2).reshape(E, H * D)),
            "wk": np.ascontiguousarray(Wk[hs].transpose(1, 0, 2).reshape(E, H * D)),
            "wv": np.ascontiguousarray(Wv[hs].transpose(1, 0, 2).reshape(E, H * D)),
            "bq": np.ascontiguousarray(bq[hs].reshape(H * D, 1)),
            "bk": np.ascontiguousarray(bk[hs].reshape(H * D, 1)),
            "bvb": np.ascontiguousarray(
                np.tile(bv[hs].reshape(1, H * D), (P, 1))),
            "wp": np.ascontiguousarray(Wp[g * H * D:(g + 1) * H * D, :]).astype(ml_dtypes.bfloat16),
            "msk": msk,
        })
    return in_maps


def gather_outputs(results, inputs):
    bp = np.asarray(inputs["bp"], np.float32)
    out = np.empty((B, T, E), np.float32)
    for b in range(B):
        out[b] = (np.asarray(results[2 * b]["y"], np.float32)
                  + np.asarray(results[2 * b + 1]["y"], np.float32)
                  + bp[None, :])
    return out


def _install_loud_hook():
    """Surface the real exception from the neuronx_cc PJRT callback."""
    import traceback
    from concourse import bass2jax
    try:
        import libneuronxla
    except ImportError:
        return
    orig = bass2jax.neuronx_cc_hook

    def loud(*a, **k):
        try:
            return orig(*a, **k)
        except BaseException:
            traceback.print_exc()
            raise

    if not hasattr(libneuronxla, "orig_neuronx_cc"):
        libneuronxla.orig_neuronx_cc = libneuronxla.neuronx_cc
    libneuronxla.neuronx_cc = loud
    bass2jax.install_neuronx_cc_hook = lambda: None


def run(inputs, trace=False):
    _install_loud_hook()
    nc = build()
    in_maps = shard_inputs(inputs)
    res = run_bass_kernel_spmd(nc, in_maps, core_ids=list(range(N_CORES)),
                               trace=trace)
    return gather_outputs(res.results, inputs), res


def kernel(**inputs):
    out, _ = run(inputs, trace=False)
    return out


def run_timed(inputs, reps=8):
    """Like run(), but executes the NEFF `reps`+1 times and returns
    (output, marginal_exec_seconds) — wall-clock per execution after the
    first (axon dispatch + HW), the closest available proxy for HW time
    when NTFF profiling is unavailable."""
    import time
    import jax
    import jax.numpy as jnp
    from jax.sharding import Mesh, PartitionSpec
    from jax.experimental.shard_map import shard_map
    from concourse import bass2jax, mybir as _mybir

    _install_loud_hook()
    nc = build()
    in_maps = shard_inputs(inputs)
    n_cores = N_CORES

    bass2jax.install_neuronx_cc_hook()
    partition_name = nc.partition_id_tensor.name if nc.partition_id_tensor else None
    in_names, out_names, out_avals, zero_outs = [], [], [], []
    for alloc in nc.m.functions[0].allocations:
        if not isinstance(alloc, _mybir.MemoryLocationSet):
            continue
        name = alloc.memorylocations[0].name
        if alloc.kind == "ExternalInput":
            if name != partition_name:
                in_names.append(name)
        elif alloc.kind == "ExternalOutput":
            shape = list(alloc.tensor_shape)
            np_dt = _mybir.dt.np(alloc.dtype)
            out_avals.append(jax.core.ShapedArray(shape, np_dt))
            out_names.append(name)
            zero_outs.append(np.zeros(shape, np_dt))
    n_params = len(in_names)
    n_outs = len(out_avals)
    in_names.extend(out_names)
    if partition_name is not None:
        in_names.append(partition_name)
    donate = tuple(range(n_params, n_params + n_outs))

    def _body(*args):
        operands = list(args)
        if partition_name is not None:
            operands.append(bass2jax.partition_id_tensor())
        return tuple(bass2jax._bass_exec_p.bind(
            *operands, out_avals=tuple(out_avals), in_names=tuple(in_names),
            out_names=tuple(out_names), lowering_input_output_aliases=(),
            sim_require_finite=True, sim_require_nnan=True, nc=nc))

    devices = jax.devices()[:n_cores]
    mesh = Mesh(np.asarray(devices), ("core",))
    spec = PartitionSpec("core")
    sharded = jax.jit(
        shard_map(_body, mesh=mesh,
                  in_specs=(spec,) * (n_params + n_outs),
                  out_specs=(spec,) * len(out_names),
                  check_rep=False),
        keep_unused=True)
    per_core = [[np.asarray(m[nm]) for nm in in_names[:n_params]]
                for m in in_maps]
    shard = jax.sharding.NamedSharding(mesh, spec)
    concat_in = [
        jax.device_put(
            np.concatenate([per_core[c][i] for c in range(n_cores)], axis=0),
            shard)
        for i in range(n_params)]
    dzeros = [
        jax.device_put(np.zeros((n_cores * z.shape[0], *z.shape[1:]), z.dtype),
                       shard)
        for z in zero_outs]

    out_arrs = sharded(*concat_in, *dzeros)  # compile + first run
    jax.block_until_ready(out_arrs)
    t0 = time.time()
    for _ in range(reps):
        r = sharded(*concat_in, *dzeros)
        jax.block_until_ready(r)
    dt = (time.time() - t0) / reps
    results = [
        {name: np.asarray(out_arrs[i]).reshape(n_cores, *out_avals[i].shape)[c]
         for i, name in enumerate(out_names)}
        for c in range(n_cores)]
    return gather_outputs(results, inputs), dt



# revision 18
# speedup vs baseline: 253.4698x; 253.4698x over previous
"""Multi-head causal attention on 8 Trainium2 NeuronCores.

Problem (hardcoded): batch_x [4, 2048, 1024], 16 heads x 64 head_size,
stacked per-head QKV params, causal softmax attention, output projection.

Sharding: 8 cores = 4 batches x 2 head-groups (8 heads each).  Each core
computes, for its (batch, head-group):
    QT/KT [hd=512, T] and V [T, hd=512] projections,
    ST = K @ Q^T per head (scores transposed: s on partitions, t free),
    P = exp(ST/8) with causal masking (upper s-blocks skipped entirely,
        diagonal blocks multiplied post-exp by a host-provided 0/1 mask),
    OT = V'^T @ P accumulated over s-blocks, where V' = [V | ones] so the
        softmax denominator accumulates in PSUM row 64 (M=65 matmuls),
    OT_norm = OT * (1/den)  (VectorE reciprocal + GPSIMD partition bcast),
    y_partial = OT_norm^T @ Wp_rows  (row-sharded output projection).
Host sums the two partials per batch and adds bp.

Dtypes: bf16 end-to-end on the PE (x, Wq/Wk/Wv, qt/kt, pt, vp, otn, Wp,
y); PSUM accumulation is always f32.  Measured on HW (unrolled-NEFF slope
method): a bf16 matmul with changing stationary weights costs ~157ns at
N=512 vs ~335ns for float32r (4-byte weight reload adds ~122ns/matmul;
bf16 reloads are free), so bf16 roughly halves real PE time for the
QKV-projection and score phases vs the f32r baseline.  Rel err ~4e-3 vs
the 2e-2 gate.

Layouts: x and every weight arrive host-pre-arranged in their exact SBUF
layouts so each DMA row is per-partition contiguous (8 KB rows; measured
~4x better effective DMA throughput than 1-2 KB rearrange rows).  Wq/Wk
are SBUF-resident (loaded once, not per chunk); bq/bk load as one [128,8]
tile instead of 512 4-byte descriptor rows.

Schedule: software-pipelined for PE occupancy (TimelineSim ~250us/core vs
~317us for the naive phase-sequential schedule):
  - exp() on the scalar engine is the per-block critical resource during
    attention (~170us busy with bf16 matmuls), so projection matmuls for
    chunk c+1 and delayed output projections are emitted as "filler" PE
    work between attention blocks (outproj(0) fills window 1;
    outproj(1)+(2) fill the Act-heaviest window 3; remainder drains at
    window end), and the QK projection moves run on DVE to stay off Act.
  - PV matmuls lag scores by PV_LAG blocks so exp/mask latency and the
    previous pair's normalize chain stay off the PE critical path; each
    pair runs its masked diagonal blocks first.
  - fillers are drained ahead of each pair's normalize so PSUM-recycling
    moves aren't stuck behind it on the in-order DVE queue.
  - the final output projection runs 6 PSUM banks wide (reusing the done
    ot/st pool banks), splits its last-pair matmuls K=64 to read the
    un-shifted head-b tile, and alternates result copies Act/DVE.
  - first-chunk input DMAs (split small) precede the one-time weight
    loads so the PE starts ~5us in instead of ~26us.
"""

import numpy as np
from collections import deque
from contextlib import ExitStack

import concourse.bass as bass
import concourse.bacc as bacc
import concourse.mybir as mybir
import concourse.tile as tile
from concourse import library_config
from concourse.bass_utils import run_bass_kernel_spmd

# problem shape (hardcoded per contest rules)
B = 4
T = 2048
E = 1024
NH = 16          # total heads
D = 64           # head size
SCALE = 1.0 / 8.0  # 1/sqrt(64)

# per-core decomposition
H = 8            # heads per core
NPAIR = 4        # head pairs per core
TCH = 512        # t-chunk (matmul free dim)
NTCH = T // TCH  # 4
P = 128
ECH = E // P     # 8 e-chunks
NSB = T // P     # 16 s-blocks
N_CORES = 8
PV_LAG = 8       # blocks between score emission and PV emission

F32 = mybir.dt.float32
F32R = mybir.dt.float32r
BF16 = mybir.dt.bfloat16
AF = mybir.ActivationFunctionType
ALU = mybir.AluOpType


def _emit(nc, tc, io):
    xT, wq, wk, wv, bqk, bvb, wp, msk, y = (
        io["xT"], io["wq"], io["wk"], io["wv"], io["bqk"],
        io["bvb"], io["wp"], io["msk"], io["y"])

    ctx = ExitStack()
    with ctx:
        # ---- resident SBUF pools (bufs=1) ----
        res = ctx.enter_context(tc.tile_pool(name="res", bufs=1))
        kt_all = res.tile([P, NPAIR, T], BF16)          # KT: partitions = pair-hd
        vp_all = res.tile([P, NSB, H, D + 1], BF16)     # V' = [V | ones]
        wv_sb = res.tile([P, ECH, TCH], BF16)
        # resident QK weights, loaded once per e-chunk (all pairs wide)
        wq_sb = res.tile([P, ECH, NPAIR * P], BF16)
        wk_sb = res.tile([P, ECH, NPAIR * P], BF16)
        wp_sb = res.tile([P, NPAIR, 2, TCH], BF16)
        bqk_sb = res.tile([P, 2, NPAIR], F32)           # [:,0,:]=bq [:,1,:]=bk
        bv_sb = res.tile([P, TCH], F32)
        msk_sb = res.tile([P, TCH], BF16)
        # pair-3 head-b rows of Wp duplicated at partitions 0-63 for the
        # final outproj's K=64 split (otb lives on partitions 0-63)
        wp_hib = res.tile([D, 2, TCH], BF16)

        # ---- cycling pools ----
        xe_pool = ctx.enter_context(tc.tile_pool(name="xe", bufs=2))
        qt_pool = ctx.enter_context(tc.tile_pool(name="qt", bufs=2))
        pt_pool = ctx.enter_context(tc.tile_pool(name="pt", bufs=10))
        otn_pool = ctx.enter_context(tc.tile_pool(name="otn", bufs=12))
        rden_pool = ctx.enter_context(tc.tile_pool(name="rden", bufs=2))
        bc_pool = ctx.enter_context(tc.tile_pool(name="bc", bufs=1))
        otb_pool = ctx.enter_context(tc.tile_pool(name="otb", bufs=1))
        ysb_pool = ctx.enter_context(tc.tile_pool(name="ysb", bufs=4))
        ps512 = ctx.enter_context(tc.tile_pool(name="ps512", bufs=2, space="PSUM"))
        st_pool = ctx.enter_context(tc.tile_pool(name="stp", bufs=2, space="PSUM"))
        ot_pool = ctx.enter_context(tc.tile_pool(name="otp", bufs=2, space="PSUM"))

        dma = nc.sync.dma_start
        nc.gpsimd.load_library(library_config.attnmlp)

        xeg = {}   # c -> xe tile [P, ECH, TCH]
        qt_c = {}  # c -> qt tile [P, NPAIR, TCH]
        otn_all = {}  # (c, p) -> otn tile [P, TCH]
        otb_last = [None]  # un-shifted head-b tile of the very last pair

        def emit_xe_dma(c):
            xt = xe_pool.tile([P, ECH, TCH], BF16, name=f"xe{c}", tag="xe")
            dma(xt, xT[c])
            xeg[c] = xt

        def load_wqk(proj, es):
            # one [P, |es|, 512] load covers e-chunks `es` for all 4 pairs;
            # host pre-arranged to SBUF layout, so rows are contiguous
            wsb = (wq_sb, wk_sb)[proj]
            src = (wq, wk)[proj]
            dma(wsb[:, es[0]:es[-1] + 1, :], src[:, es[0]:es[-1] + 1, :])

        def proj_closures(c):
            """Closure list for chunk c's QKV projections, in emission order
            [q(p0), k(p0), V(i0..3), q(p1), k(p1), ..., q(p3), k(p3)] — the
            prologue drains the first 6 chains dense, window fillers take the
            rest.  Each chain is 8 matmuls + a move w/ bias.  QK weights are
            resident (wq_sb/wk_sb, loaded once in the prologue)."""
            t0 = c * TCH

            def xe_rhs(e):
                return xeg[c][:, e, :]

            def xe_lhs(e, i):
                return xeg[c][:, e, i * P:(i + 1) * P]

            qt_c[c] = qt_pool.tile([P, NPAIR, TCH], BF16, name=f"qtc{c}",
                                   tag="qt")
            chains = []  # (proj, p, dest, label) per q/k chain
            for p in range(NPAIR):
                chains.append((0, p, qt_c[c][:, p, :], f"q{c}_{p}"))
                chains.append((1, p, kt_all[:, p, t0:t0 + TCH],
                               f"k{c}_{p}"))

            def qk_ops(n):
                ops = []
                proj, p, dest, label = chains[n]
                wsb = (wq_sb, wk_sb)[proj]
                ps = ps512.tile([P, TCH], F32, name=f"ps{label}", tag="ps512")
                for e in range(ECH):
                    def mm(proj=proj, p=p, ps=ps, e=e, wsb=wsb):
                        nc.tensor.matmul(ps[:], wsb[:, e, p * P:(p + 1) * P],
                                         xe_rhs(e),
                                         start=(e == 0), stop=(e == ECH - 1),
                                         skip_group_check=True)
                    ops.append(mm)

                def mv(proj=proj, p=p, dest=dest, ps=ps):
                    # on DVE: with bf16 matmuls the Act engine's exp stream is
                    # the pacing resource (~170us busy), so these moves must
                    # stay off it; DVE has ~2x headroom
                    nc.vector.tensor_add(
                        dest, ps[:],
                        bqk_sb[:, proj, p:p + 1].broadcast_to([P, TCH]))
                ops.append(mv)
                return ops

            def v_ops(i):
                ops = []
                tt = 4 * c + i
                ps = ps512.tile([P, TCH], F32, name=f"v{c}_{i}", tag="ps512")
                for e in range(ECH):
                    def mm(ps=ps, e=e, i=i):
                        nc.tensor.matmul(ps[:], xe_lhs(e, i), wv_sb[:, e, :],
                                         start=(e == 0), stop=(e == ECH - 1),
                                         skip_group_check=True)
                    ops.append(mm)

                def mv(ps=ps, tt=tt):
                    nc.vector.tensor_add(
                        vp_all[:, tt, :, 0:D],
                        ps[:].rearrange("p (h d) -> p h d", d=D),
                        bv_sb[:].rearrange("p (h d) -> p h d", d=D))
                ops.append(mv)
                return ops

            ops = qk_ops(0) + qk_ops(1)
            for i in range(4):
                ops += v_ops(i)
            for n in range(2, len(chains)):
                ops += qk_ops(n)
            return ops

        def outproj_closures(cc):
            """Filler closures for the output projection of chunk cc."""
            ops = []
            for i in range(4):
                tt = 4 * cc + i
                ysb = ysb_pool.tile([P, 2, TCH], BF16, name=f"ysb{cc}_{i}",
                                    tag="ysb")
                for ec in range(2):
                    yp = ps512.tile([P, TCH], F32, name=f"y{cc}_{i}_{ec}",
                                    tag="ps512")
                    for p in range(NPAIR):
                        def mm(cc=cc, i=i, ec=ec, p=p, yp=yp):
                            nc.tensor.matmul(
                                yp[:], otn_all[(cc, p)][:, i * P:(i + 1) * P],
                                wp_sb[:, p, ec, :],
                                start=(p == 0), stop=(p == NPAIR - 1),
                                skip_group_check=True)
                        ops.append(mm)

                    def cp(ysb=ysb, ec=ec, yp=yp):
                        nc.vector.tensor_copy(ysb[:, ec, :], yp[:])
                    ops.append(cp)

                def st(tt=tt, ysb=ysb):
                    dma(y[tt * P:(tt + 1) * P, :],
                        ysb[:].rearrange("p a b -> p (a b)"))
                ops.append(st)
            return ops

        def normalize(c, p, ot_a, ot_b):
            rden_a = rden_pool.tile([1, TCH], F32, name=f"rda{c}_{p}", tag="rd")
            rden_b = rden_pool.tile([1, TCH], F32, name=f"rdb{c}_{p}", tag="rd")
            nc.vector.reciprocal(rden_a[:], ot_a[D:D + 1, :])
            nc.vector.reciprocal(rden_b[:], ot_b[D:D + 1, :])
            bc_a = bc_pool.tile([D, TCH], F32, name=f"bca{c}_{p}", tag="bca")
            bc_b = bc_pool.tile([D, TCH], F32, name=f"bcb{c}_{p}", tag="bcb")
            nc.gpsimd.partition_broadcast(bc_a[:], rden_a[:], channels=D)
            nc.gpsimd.partition_broadcast(bc_b[:], rden_b[:], channels=D)
            otn = otn_pool.tile([P, TCH], BF16, name=f"otn{c}_{p}", tag="otn")
            otb = otb_pool.tile([D, TCH], BF16, name=f"otb{c}_{p}", tag="otb")
            nc.vector.tensor_mul(otn[0:D, :], ot_a[0:D, :], bc_a[:])
            nc.vector.tensor_mul(otb[:], ot_b[0:D, :], bc_b[:])
            if (c, p) == (NTCH - 1, NPAIR - 1):
                # last pair of the kernel: the final outproj reads otb
                # directly via a K=64 matmul split, so the ~2us DMA shift
                # latency stays off the critical tail
                otb_last[0] = otb
            else:
                # partition shift 0:64 -> 64:128 (DMA; DVE lanes can't shift)
                dma(otn[D:2 * D, :], otb[:])
            otn_all[(c, p)] = otn

        def window(c, fillers, guards=None):
            """Attention for chunk c, interleaved with filler PE work.

            guards[p] = minimum number of fillers that must be EMITTED before
            pair p's first score (Tile versioning is emission-ordered, so a
            score reading qt/kt written by a not-yet-emitted filler move would
            silently read the previous version)."""
            nb = 4 * (c + 1)  # causal s-blocks for this chunk
            popped = [0]

            def pop_filler():
                if fillers:
                    fillers.popleft()()
                    popped[0] += 1

            for p in range(NPAIR):
                if guards:
                    while popped[0] < guards.get(p, 0) and fillers:
                        pop_filler()
                ot_a = ot_pool.tile([D + 1, TCH], F32, name=f"ota{c}_{p}",
                                    tag="ot")
                ot_b = ot_pool.tile([D + 1, TCH], F32, name=f"otb{c}_{p}",
                                    tag="ot")
                pv_q = deque()
                # diagonal blocks first: their mask multiplies land on the
                # in-order DVE queue while it still has the previous pair's
                # normalize queued, and the PV lag covers both; the unmasked
                # wide blocks then flush the pair densely.  Pair 0 keeps them
                # last — at window start DVE is busy with boundary moves.
                ks = list(range(4 * c, nb)) + list(range(0, 4 * c))
                for ki, k in enumerate(ks):
                    # diagonal blocks: columns < 128j are fully masked; trim
                    # all work to the live range [t1:TCH] (t1 = 128j).
                    j = k - 4 * c
                    t1 = 128 * j if j > 0 else 0
                    w = TCH - t1
                    st = st_pool.tile([P, 2, TCH], F32, name=f"st{c}_{p}_{k}",
                                      tag="st")
                    pt = pt_pool.tile([P, 2, TCH], BF16, name=f"pt{c}_{p}_{k}",
                                      tag="pt")
                    for h in (0, 1):
                        lo = 64 * h
                        nc.tensor.matmul(
                            st[:, h, t1:TCH],
                            kt_all[lo:lo + 64, p, k * P:(k + 1) * P],
                            qt_c[c][lo:lo + 64, p, t1:TCH],
                            start=True, stop=True)
                    nc.scalar.activation(pt[:, :, t1:TCH], st[:, :, t1:TCH],
                                         AF.Exp, scale=SCALE)
                    if j >= 0:
                        # the mask is only != 1 inside the 128-wide diagonal
                        # square [t1, t1+128): beyond it t >= 128j+s for all
                        # s, so the multiply needn't touch those columns
                        nc.vector.tensor_mul(
                            pt[:, :, t1:t1 + P], pt[:, :, t1:t1 + P],
                            msk_sb[:, 0:P].rearrange("p (a w) -> p a w", a=1)
                            .broadcast_to([P, 2, P]))

                    def pv(k=k, ki=ki, t1=t1, pt=pt, ot_a=ot_a, ot_b=ot_b):
                        st_flag = (ki == 0)
                        sp_flag = (ki == nb - 1)
                        nc.tensor.matmul(ot_a[:, t1:TCH],
                                         vp_all[:, k, 2 * p, :],
                                         pt[:, 0, t1:TCH],
                                         start=st_flag, stop=sp_flag,
                                         skip_group_check=True)
                        nc.tensor.matmul(ot_b[:, t1:TCH],
                                         vp_all[:, k, 2 * p + 1, :],
                                         pt[:, 1, t1:TCH],
                                         start=st_flag, stop=sp_flag,
                                         skip_group_check=True)
                    pv_q.append(pv)
                    if len(pv_q) > PV_LAG:
                        pv_q.popleft()()
                    pop_filler()
                while pv_q:
                    pv_q.popleft()()
                # let filler moves/copies go ahead of this pair's normalize on
                # the in-order DVE queue — they recycle PSUM banks that pending
                # PE matmuls wait on (drain everything before the last pair)
                for _ in range(6 if p < NPAIR - 1 else len(fillers)):
                    pop_filler()
                normalize(c, p, ot_a, ot_b)
            # drain remaining fillers before the next chunk's attention
            while fillers:
                fillers.popleft()()

        # ================= prologue =================
        # First-chunk inputs first so the PE can start ASAP; DMAs fire in
        # consumption order so the first matmul's two dependencies (xe
        # e0-slice + wq e0-slice, 128 KB each) sit at the queue head.  QK
        # weights load once here (resident for all chunks); wp defers into
        # window 0's fillers.
        xv0 = xT[0]
        xt0 = xe_pool.tile([P, ECH, TCH], BF16, name="xe0", tag="xe")
        xeg[0] = xt0
        dma(xt0[:, 0, :], xv0[:, 0, :])              # x e-chunk 0
        fill0 = deque(proj_closures(0))
        load_wqk(0, [0])                             # wq e0
        dma(xt0[:, 1, :], xv0[:, 1, :])              # x e-chunk 1
        load_wqk(0, [1])                             # wq e1
        dma(xt0[:, 2:4, :], xv0[:, 2:4, :])          # x e2,3
        load_wqk(0, [2, 3])                          # wq e2,3
        dma(xt0[:, 4:8, :], xv0[:, 4:8, :])          # x e4..7
        load_wqk(0, [4, 5, 6, 7])                    # wq e4..7
        load_wqk(1, [0, 1, 2, 3])                    # wk e0..3
        load_wqk(1, [4, 5, 6, 7])                    # wk e4..7
        dma(bqk_sb[:], bqk.rearrange("p (a b) -> p a b", a=2))
        dma(bv_sb[:], bvb[:, :])
        for g in range(4):
            dma(wv_sb[:, 2 * g:2 * g + 2, :], wv[:, 2 * g:2 * g + 2, :])
        dma(msk_sb[:], msk[:, :])
        nc.vector.memset(vp_all[:, :, :, D:D + 1], 1.0)
        for _ in range(6 * 9):  # q(p0), k(p0), V(i0..3) dense
            fill0.popleft()()

        def load_wp():
            dma(wp_sb[:], wp)
            dma(wp_hib[:], wp[D:P, NPAIR - 1])

        # ================= main windows =================
        # outproj(0) fills window 1; outproj(1)+(2) fill window 3, which has
        # no projection work left but the most attention (Act-bound) blocks
        op_sched = {1: (0,), 2: (), 3: (1, 2)}
        for c in range(NTCH):
            fillers = deque()
            guards = None
            if c == 0:
                fillers.extend(fill0)  # q/k chains for pairs 1..3
                fillers.append(load_wp)
                guards = {1: 18, 2: 36, 3: 54}
            for cc in op_sched.get(c, ()):
                fillers.extend(outproj_closures(cc))
            if c + 1 < NTCH:
                emit_xe_dma(c + 1)
                fillers.extend(proj_closures(c + 1))
            window(c, fillers, guards)

        # ============ final output projection (chunk 3) ============
        # Emitted so PE never head-of-line blocks on the last pair's
        # normalize: p0-p2 accumulations (ready) go first two chains at a
        # time; each chain's p3 is split into two K=64 matmuls (otn head-a +
        # un-shifted otb head-b); result copies ride the now-idle Act engine.
        cc = NTCH - 1
        chains = [(i, ec) for i in range(4) for ec in range(2)]
        yps = {}
        ysbs = {}
        # attention is over: the ot/st pools' PSUM banks are free and their
        # slot sizes fit a [P, TCH] f32 accumulator, so the final outproj can
        # run 6 chains in flight instead of 2
        # ot slots last: their previous occupants (the final pair's ot_a/b)
        # are only freed by the last normalize's reads
        yf_pools = [(ps512, "ps512"), (ps512, "ps512"), (st_pool, "st"),
                    (st_pool, "st"), (ot_pool, "ot"), (ot_pool, "ot")]

        def p012(n):
            i, ec = chains[n]
            pool, tag = yf_pools[n % len(yf_pools)]
            yp = pool.tile([P, TCH], F32, name=f"yf{i}_{ec}", tag=tag)
            yps[n] = yp
            for p in range(3):
                nc.tensor.matmul(yp[:], otn_all[(cc, p)][:, i * P:(i + 1) * P],
                                 wp_sb[:, p, ec, :], start=(p == 0),
                                 stop=False, skip_group_check=True)

        def p3_and_copy(n):
            i, ec = chains[n]
            yp = yps[n]
            otn3 = otn_all[(cc, NPAIR - 1)]
            otb3 = otb_last[0]
            nc.tensor.matmul(yp[:], otn3[0:D, i * P:(i + 1) * P],
                             wp_sb[0:D, NPAIR - 1, ec, :],
                             start=False, stop=False, skip_group_check=True)
            nc.tensor.matmul(yp[:], otb3[:, i * P:(i + 1) * P],
                             wp_hib[:, ec, :],
                             start=False, stop=True, skip_group_check=True)
            if i not in ysbs:
                ysbs[i] = ysb_pool.tile([P, 2, TCH], BF16, name=f"ysbf{i}",
                                        tag="ysb")
            if n % 2 == 0:
                nc.scalar.activation(ysbs[i][:, ec, :], yp[:], AF.Copy)
            else:
                nc.vector.tensor_copy(ysbs[i][:, ec, :], yp[:])
            if ec == 1:
                tt = 4 * cc + i
                dma(y[tt * P:(tt + 1) * P, :],
                    ysbs[i][:].rearrange("p a b -> p (a b)"))

        for n in range(6):
            p012(n)
        for n in range(len(chains)):
            p3_and_copy(n)
            if n + 6 < len(chains):
                p012(n + 6)


def make_io(nc):
    # x and all weights arrive host-pre-arranged in their exact SBUF layouts
    # so every DMA row is per-partition contiguous (8 KB rows — measured ~4x
    # better effective DMA throughput than the 1-2 KB rows a DRAM-side
    # rearrange produces)
    return {
        "xT": nc.dram_tensor("xT", [NTCH, P, ECH, TCH], BF16,
                             kind="ExternalInput").ap(),
        "wq": nc.dram_tensor("wq", [P, ECH, TCH], BF16,
                             kind="ExternalInput").ap(),
        "wk": nc.dram_tensor("wk", [P, ECH, TCH], BF16,
                             kind="ExternalInput").ap(),
        "wv": nc.dram_tensor("wv", [P, ECH, TCH], BF16,
                             kind="ExternalInput").ap(),
        "bqk": nc.dram_tensor("bqk", [P, 2 * NPAIR], F32,
                              kind="ExternalInput").ap(),
        "bvb": nc.dram_tensor("bvb", [P, H * D], F32, kind="ExternalInput").ap(),
        "wp": nc.dram_tensor("wp", [P, NPAIR, 2, TCH], BF16,
                             kind="ExternalInput").ap(),
        "msk": nc.dram_tensor("msk", [P, TCH], BF16, kind="ExternalInput").ap(),
        "y": nc.dram_tensor("y", [T, E], BF16, kind="ExternalOutput").ap(),
    }


def build(reps=1):
    """reps>1 emits the body multiple times into one NEFF (used by test.py's
    slope-based timing; kernel() always uses reps=1)."""
    nc = bacc.Bacc(trn_type="TRN2", target_bir_lowering=False, debug=False)
    io = make_io(nc)
    with tile.TileContext(nc) as tc:
        for _ in range(reps):
            _emit(nc, tc, io)
    nc.compile()
    return nc


def shard_inputs(inputs):
    """Full inputs -> per-core in_maps (8 cores: batch-major, group-minor)."""
    bx = np.asarray(inputs["batch_x"], np.float32)
    Wq = np.asarray(inputs["Wq"], np.float32)
    Wk = np.asarray(inputs["Wk"], np.float32)
    Wv = np.asarray(inputs["Wv"], np.float32)
    bq = np.asarray(inputs["bq"], np.float32)
    bk = np.asarray(inputs["bk"], np.float32)
    bv = np.asarray(inputs["bv"], np.float32)
    Wp = np.asarray(inputs["Wp"], np.float32)

    import ml_dtypes
    bf16 = ml_dtypes.bfloat16
    ps = np.arange(P, dtype=np.float32)[:, None]
    tf = np.arange(TCH, dtype=np.float32)[None, :]
    msk = (tf >= ps).astype(bf16)

    in_maps = []
    for core in range(N_CORES):
        b, g = core // 2, core % 2
        hs = slice(g * H, (g + 1) * H)
        bqf = bq[hs].reshape(NPAIR, P).T          # [128, 4] pair-major
        bkf = bk[hs].reshape(NPAIR, P).T

        def sb_w(W):
            # [E, H*D] -> SBUF layout [r=128, e-chunk=8, m=512]
            We = W[hs].transpose(1, 0, 2).reshape(E, H * D)
            return np.ascontiguousarray(
                We.reshape(ECH, P, H * D).transpose(1, 0, 2)).astype(bf16)

        xT = bx[b].T                              # [E, T]
        xh = np.ascontiguousarray(
            xT.reshape(ECH, P, NTCH, TCH).transpose(2, 1, 0, 3)).astype(bf16)
        Wpg = Wp[g * H * D:(g + 1) * H * D, :]    # [512, 1024]
        wph = np.ascontiguousarray(
            Wpg.reshape(NPAIR, P, 2, TCH).transpose(1, 0, 2, 3)).astype(bf16)
        in_maps.append({
            "xT": xh,
            "wq": sb_w(Wq),
            "wk": sb_w(Wk),
            "wv": sb_w(Wv),
            "bqk": np.ascontiguousarray(
                np.concatenate([bqf, bkf], axis=1)),
            "bvb": np.ascontiguousarray(
                np.tile(bv[hs].reshape(1, H * D), (P, 1))),
            "wp": wph,
            "msk": msk,
        })
    return in_maps


def gather_outputs(results, inputs):
    bp = np.asarray(inputs["bp"], np.float32)
    out = np.empty((B, T, E), np.float32)
    for b in range(B):
        out[b] = (np.asarray(results[2 * b]["y"], np.float32)
                  + np.asarray(results[2 * b + 1]["y"], np.float32)
                  + bp[None, :])
    return out


def _install_loud_hook():
    """Surface the real exception from the neuronx_cc PJRT callback."""
    import traceback
    from concourse import bass2jax
    try:
        import libneuronxla
    except ImportError:
        return
    orig = bass2jax.neuronx_cc_hook

    def loud(*a, **k):
        try:
            return orig(*a, **k)
        except BaseException:
            traceback.print_exc()
            raise

    if not hasattr(libneuronxla, "orig_neuronx_cc"):
        libneuronxla.orig_neuronx_cc = libneuronxla.neuronx_cc
    libneuronxla.neuronx_cc = loud
    bass2jax.install_neuronx_cc_hook = lambda: None


_NC_CACHE = []


def run(inputs, trace=False):
    _install_loud_hook()
    if not _NC_CACHE:
        _NC_CACHE.append(build())
    nc = _NC_CACHE[0]
    in_maps = shard_inputs(inputs)
    res = run_bass_kernel_spmd(nc, in_maps, core_ids=list(range(N_CORES)),
                               trace=trace)
    return gather_outputs(res.results, inputs), res


def kernel(**inputs):
    out, _ = run(inputs, trace=False)
    return out


# revision 25
# speedup vs baseline: 256.7924x; 1.0131x over previous
"""Multi-head causal attention on 8 Trainium2 NeuronCores.

Problem (hardcoded): batch_x [4, 2048, 1024], 16 heads x 64 head_size,
stacked per-head QKV params, causal softmax attention, output projection.

Sharding: 8 cores = 4 batches x 2 head-groups (8 heads each).  Each core
computes, for its (batch, head-group):
    QT/KT [hd=512, T] and V [T, hd=512] projections,
    ST = K @ Q^T per head (scores transposed: s on partitions, t free),
    P = exp(ST/8) with causal masking (upper s-blocks skipped entirely,
        diagonal blocks multiplied post-exp by a host-provided 0/1 mask),
    OT = V'^T @ P accumulated over s-blocks, where V' = [V | ones] so the
        softmax denominator accumulates in PSUM row 64 (M=65 matmuls),
    OT_norm = OT * (1/den)  (VectorE reciprocal + GPSIMD partition bcast),
    y_partial = OT_norm^T @ Wp_rows  (row-sharded output projection).
Host sums the two partials per batch and adds bp.

Dtypes: bf16 end-to-end on the PE (x, Wq/Wk/Wv, qt/kt, pt, vp, otn, Wp,
y); PSUM accumulation is always f32.  Measured on HW (unrolled-NEFF slope
method): a bf16 matmul with changing stationary weights costs ~157ns at
N=512 vs ~335ns for float32r (4-byte weight reload adds ~122ns/matmul;
bf16 reloads are free), so bf16 roughly halves real PE time for the
QKV-projection and score phases vs the f32r baseline.  Rel err ~4e-3 vs
the 2e-2 gate.

Layouts: x and every weight arrive host-pre-arranged in their exact SBUF
layouts so each DMA row is per-partition contiguous (8 KB rows; measured
~4x better effective DMA throughput than 1-2 KB rearrange rows).  Wq/Wk
are SBUF-resident (loaded once, not per chunk); bq/bk load as one [128,8]
tile instead of 512 4-byte descriptor rows.

Schedule: software-pipelined for PE occupancy (TimelineSim ~250us/core vs
~317us for the naive phase-sequential schedule):
  - exp() on the scalar engine is the per-block critical resource during
    attention (~170us busy with bf16 matmuls), so projection matmuls for
    chunk c+1 and delayed output projections are emitted as "filler" PE
    work between attention blocks (outproj(0) fills window 1;
    outproj(1)+(2) fill the Act-heaviest window 3; remainder drains at
    window end), and the QK projection moves run on DVE to stay off Act.
  - PV matmuls lag scores by PV_LAG blocks so exp/mask latency and the
    previous pair's normalize chain stay off the PE critical path; each
    pair runs its masked diagonal blocks first.
  - fillers are paced evenly over the window's remaining blocks (a lump
    draining densely at window end stalls the exp stream ~9-13us per
    boundary), and the QK projection moves alternate Act/DVE so the
    ps512 PSUM recycling never queues deep behind either engine.
    (Measured dead ends, kept for the record: evacuating the ot PSUM
    banks to SBUF before the normalize chain costs more on the DVE
    queue than it saves; gpsimd partition_broadcast is ~2.5us/call on
    real HW but sits far enough off the critical path.)
  - fillers are drained ahead of each pair's normalize so PSUM-recycling
    moves aren't stuck behind it on the in-order DVE queue.
  - the final output projection runs 6 PSUM banks wide (reusing the done
    ot/st pool banks), splits its last-pair matmuls K=64 to read the
    un-shifted head-b tile, and alternates result copies Act/DVE.
  - first-chunk input DMAs (split small) precede the one-time weight
    loads so the PE starts ~5us in instead of ~26us.
"""

import numpy as np
from collections import deque
from contextlib import ExitStack

import concourse.bass as bass
import concourse.bacc as bacc
import concourse.mybir as mybir
import concourse.tile as tile
from concourse import library_config
from concourse.bass_utils import run_bass_kernel_spmd

# problem shape (hardcoded per contest rules)
B = 4
T = 2048
E = 1024
NH = 16          # total heads
D = 64           # head size
SCALE = 1.0 / 8.0  # 1/sqrt(64)

# per-core decomposition
H = 8            # heads per core
NPAIR = 4        # head pairs per core
TCH = 512        # t-chunk (matmul free dim)
NTCH = T // TCH  # 4
P = 128
ECH = E // P     # 8 e-chunks
NSB = T // P     # 16 s-blocks
N_CORES = 8
PV_LAG = 8       # blocks between score emission and PV emission

F32 = mybir.dt.float32
F32R = mybir.dt.float32r
BF16 = mybir.dt.bfloat16
AF = mybir.ActivationFunctionType
ALU = mybir.AluOpType


def _emit(nc, tc, io):
    xT, wq, wk, wv, bqk, bvb, wp, msk, y = (
        io["xT"], io["wq"], io["wk"], io["wv"], io["bqk"],
        io["bvb"], io["wp"], io["msk"], io["y"])

    ctx = ExitStack()
    with ctx:
        # ---- resident SBUF pools (bufs=1) ----
        res = ctx.enter_context(tc.tile_pool(name="res", bufs=1))
        kt_all = res.tile([P, NPAIR, T], BF16)          # KT: partitions = pair-hd
        vp_all = res.tile([P, NSB, H, D + 1], BF16)     # V' = [V | ones]
        wv_sb = res.tile([P, ECH, TCH], BF16)
        # resident QK weights, loaded once per e-chunk (all pairs wide)
        wq_sb = res.tile([P, ECH, NPAIR * P], BF16)
        wk_sb = res.tile([P, ECH, NPAIR * P], BF16)
        wp_sb = res.tile([P, NPAIR, 2, TCH], BF16)
        bqk_sb = res.tile([P, 2, NPAIR], F32)           # [:,0,:]=bq [:,1,:]=bk
        bv_sb = res.tile([P, TCH], F32)
        msk_sb = res.tile([P, TCH], BF16)
        # pair-3 head-b rows of Wp duplicated at partitions 0-63 for the
        # final outproj's K=64 split (otb lives on partitions 0-63)
        wp_hib = res.tile([D, 2, TCH], BF16)

        # ---- cycling pools ----
        xe_pool = ctx.enter_context(tc.tile_pool(name="xe", bufs=2))
        qt_pool = ctx.enter_context(tc.tile_pool(name="qt", bufs=2))
        pt_pool = ctx.enter_context(tc.tile_pool(name="pt", bufs=10))
        otn_pool = ctx.enter_context(tc.tile_pool(name="otn", bufs=12))
        rden_pool = ctx.enter_context(tc.tile_pool(name="rden", bufs=2))
        bc_pool = ctx.enter_context(tc.tile_pool(name="bc", bufs=1))
        otb_pool = ctx.enter_context(tc.tile_pool(name="otb", bufs=1))
        ysb_pool = ctx.enter_context(tc.tile_pool(name="ysb", bufs=4))
        ps512 = ctx.enter_context(tc.tile_pool(name="ps512", bufs=2, space="PSUM"))
        st_pool = ctx.enter_context(tc.tile_pool(name="stp", bufs=2, space="PSUM"))
        ot_pool = ctx.enter_context(tc.tile_pool(name="otp", bufs=2, space="PSUM"))

        dma = nc.sync.dma_start
        nc.gpsimd.load_library(library_config.attnmlp)

        xeg = {}   # c -> xe tile [P, ECH, TCH]
        qt_c = {}  # c -> qt tile [P, NPAIR, TCH]
        otn_all = {}  # (c, p) -> otn tile [P, TCH]
        otb_last = [None]  # un-shifted head-b tile of the very last pair

        def emit_xe_dma(c):
            xt = xe_pool.tile([P, ECH, TCH], BF16, name=f"xe{c}", tag="xe")
            dma(xt, xT[c])
            xeg[c] = xt

        def load_wqk(proj, es):
            # one [P, |es|, 512] load covers e-chunks `es` for all 4 pairs;
            # host pre-arranged to SBUF layout, so rows are contiguous
            wsb = (wq_sb, wk_sb)[proj]
            src = (wq, wk)[proj]
            dma(wsb[:, es[0]:es[-1] + 1, :], src[:, es[0]:es[-1] + 1, :])

        def proj_closures(c):
            """Closure list for chunk c's QKV projections, in emission order
            [q(p0), k(p0), V(i0..3), q(p1), k(p1), ..., q(p3), k(p3)] — the
            prologue drains the first 6 chains dense, window fillers take the
            rest.  Each chain is 8 matmuls + a move w/ bias.  QK weights are
            resident (wq_sb/wk_sb, loaded once in the prologue)."""
            t0 = c * TCH

            def xe_rhs(e):
                return xeg[c][:, e, :]

            def xe_lhs(e, i):
                return xeg[c][:, e, i * P:(i + 1) * P]

            qt_c[c] = qt_pool.tile([P, NPAIR, TCH], BF16, name=f"qtc{c}",
                                   tag="qt")
            chains = []  # (proj, p, dest, label) per q/k chain
            for p in range(NPAIR):
                chains.append((0, p, qt_c[c][:, p, :], f"q{c}_{p}"))
                chains.append((1, p, kt_all[:, p, t0:t0 + TCH],
                               f"k{c}_{p}"))

            def qk_ops(n):
                ops = []
                proj, p, dest, label = chains[n]
                wsb = (wq_sb, wk_sb)[proj]
                ps = ps512.tile([P, TCH], F32, name=f"ps{label}", tag="ps512")
                for e in range(ECH):
                    def mm(proj=proj, p=p, ps=ps, e=e, wsb=wsb):
                        nc.tensor.matmul(ps[:], wsb[:, e, p * P:(p + 1) * P],
                                         xe_rhs(e),
                                         start=(e == 0), stop=(e == ECH - 1),
                                         skip_group_check=True)
                    ops.append(mm)

                def mv(n=n, proj=proj, p=p, dest=dest, ps=ps):
                    # alternate Act/DVE: these moves recycle the ps512 PSUM
                    # banks the next filler chain waits on, so queue latency
                    # on either single engine stalls the PE (model: ~1-2us
                    # per chain); splitting halves the in-order queue depth
                    if n % 2:
                        nc.scalar.activation(dest, ps[:], AF.Identity,
                                             bias=bqk_sb[:, proj, p:p + 1])
                    else:
                        nc.vector.tensor_add(
                            dest, ps[:],
                            bqk_sb[:, proj, p:p + 1].broadcast_to([P, TCH]))
                ops.append(mv)
                return ops

            def v_ops(i):
                ops = []
                tt = 4 * c + i
                ps = ps512.tile([P, TCH], F32, name=f"v{c}_{i}", tag="ps512")
                for e in range(ECH):
                    def mm(ps=ps, e=e, i=i):
                        nc.tensor.matmul(ps[:], xe_lhs(e, i), wv_sb[:, e, :],
                                         start=(e == 0), stop=(e == ECH - 1),
                                         skip_group_check=True)
                    ops.append(mm)

                def mv(ps=ps, tt=tt):
                    nc.vector.tensor_add(
                        vp_all[:, tt, :, 0:D],
                        ps[:].rearrange("p (h d) -> p h d", d=D),
                        bv_sb[:].rearrange("p (h d) -> p h d", d=D))
                ops.append(mv)
                return ops

            ops = qk_ops(0) + qk_ops(1)
            for i in range(4):
                ops += v_ops(i)
            for n in range(2, len(chains)):
                ops += qk_ops(n)
            return ops

        def outproj_closures(cc):
            """Filler closures for the output projection of chunk cc."""
            ops = []
            for i in range(4):
                tt = 4 * cc + i
                ysb = ysb_pool.tile([P, 2, TCH], BF16, name=f"ysb{cc}_{i}",
                                    tag="ysb")
                for ec in range(2):
                    yp = ps512.tile([P, TCH], F32, name=f"y{cc}_{i}_{ec}",
                                    tag="ps512")
                    for p in range(NPAIR):
                        def mm(cc=cc, i=i, ec=ec, p=p, yp=yp):
                            nc.tensor.matmul(
                                yp[:], otn_all[(cc, p)][:, i * P:(i + 1) * P],
                                wp_sb[:, p, ec, :],
                                start=(p == 0), stop=(p == NPAIR - 1),
                                skip_group_check=True)
                        ops.append(mm)

                    def cp(ysb=ysb, ec=ec, yp=yp):
                        nc.vector.tensor_copy(ysb[:, ec, :], yp[:])
                    ops.append(cp)

                def st(tt=tt, ysb=ysb):
                    dma(y[tt * P:(tt + 1) * P, :],
                        ysb[:].rearrange("p a b -> p (a b)"))
                ops.append(st)
            return ops

        def normalize(c, p, ot_a, ot_b):
            rden_a = rden_pool.tile([1, TCH], F32, name=f"rda{c}_{p}", tag="rd")
            rden_b = rden_pool.tile([1, TCH], F32, name=f"rdb{c}_{p}", tag="rd")
            nc.vector.reciprocal(rden_a[:], ot_a[D:D + 1, :])
            nc.vector.reciprocal(rden_b[:], ot_b[D:D + 1, :])
            bc_a = bc_pool.tile([D, TCH], F32, name=f"bca{c}_{p}", tag="bca")
            bc_b = bc_pool.tile([D, TCH], F32, name=f"bcb{c}_{p}", tag="bcb")
            nc.gpsimd.partition_broadcast(bc_a[:], rden_a[:], channels=D)
            nc.gpsimd.partition_broadcast(bc_b[:], rden_b[:], channels=D)
            otn = otn_pool.tile([P, TCH], BF16, name=f"otn{c}_{p}", tag="otn")
            otb = otb_pool.tile([D, TCH], BF16, name=f"otb{c}_{p}", tag="otb")
            nc.vector.tensor_mul(otn[0:D, :], ot_a[0:D, :], bc_a[:])
            nc.vector.tensor_mul(otb[:], ot_b[0:D, :], bc_b[:])
            if (c, p) == (NTCH - 1, NPAIR - 1):
                # last pair of the kernel: the final outproj reads otb
                # directly via a K=64 matmul split, so the ~2us DMA shift
                # latency stays off the critical tail
                otb_last[0] = otb
            else:
                # partition shift 0:64 -> 64:128 (DMA; DVE lanes can't shift)
                dma(otn[D:2 * D, :], otb[:])
            otn_all[(c, p)] = otn

        def window(c, fillers, guards=None):
            """Attention for chunk c, interleaved with filler PE work.

            guards[p] = minimum number of fillers that must be EMITTED before
            pair p's first score (Tile versioning is emission-ordered, so a
            score reading qt/kt written by a not-yet-emitted filler move would
            silently read the previous version)."""
            nb = 4 * (c + 1)  # causal s-blocks for this chunk
            popped = [0]
            blocks_done = [0]
            total_blocks = NPAIR * nb

            def pop_filler():
                if fillers:
                    fillers.popleft()()
                    popped[0] += 1

            def pace_fillers():
                # spread remaining fillers evenly over remaining blocks: a
                # leftover lump draining densely at window end stalls the
                # exp stream ~9-13us per boundary (next window's scores only
                # emit after the drain)
                blocks_done[0] += 1
                rem = total_blocks - blocks_done[0]
                if rem <= 0:
                    return
                per = -(-len(fillers) // rem)   # ceil
                for _ in range(per):
                    pop_filler()

            for p in range(NPAIR):
                if guards:
                    while popped[0] < guards.get(p, 0) and fillers:
                        pop_filler()
                ot_a = ot_pool.tile([D + 1, TCH], F32, name=f"ota{c}_{p}",
                                    tag="ot")
                ot_b = ot_pool.tile([D + 1, TCH], F32, name=f"otb{c}_{p}",
                                    tag="ot")
                pv_q = deque()
                # diagonal blocks first: their mask multiplies land on the
                # in-order DVE queue while it still has the previous pair's
                # normalize queued, and the PV lag covers both; the unmasked
                # wide blocks then flush the pair densely.  Pair 0 keeps them
                # last — at window start DVE is busy with boundary moves.
                ks = list(range(4 * c, nb)) + list(range(0, 4 * c))
                for ki, k in enumerate(ks):
                    # diagonal blocks: columns < 128j are fully masked; trim
                    # all work to the live range [t1:TCH] (t1 = 128j).
                    j = k - 4 * c
                    t1 = 128 * j if j > 0 else 0
                    w = TCH - t1
                    st = st_pool.tile([P, 2, TCH], F32, name=f"st{c}_{p}_{k}",
                                      tag="st")
                    pt = pt_pool.tile([P, 2, TCH], BF16, name=f"pt{c}_{p}_{k}",
                                      tag="pt")
                    for h in (0, 1):
                        lo = 64 * h
                        nc.tensor.matmul(
                            st[:, h, t1:TCH],
                            kt_all[lo:lo + 64, p, k * P:(k + 1) * P],
                            qt_c[c][lo:lo + 64, p, t1:TCH],
                            start=True, stop=True)
                    nc.scalar.activation(pt[:, :, t1:TCH], st[:, :, t1:TCH],
                                         AF.Exp, scale=SCALE)
                    if j >= 0:
                        # the mask is only != 1 inside the 128-wide diagonal
                        # square [t1, t1+128): beyond it t >= 128j+s for all
                        # s, so the multiply needn't touch those columns
                        nc.vector.tensor_mul(
                            pt[:, :, t1:t1 + P], pt[:, :, t1:t1 + P],
                            msk_sb[:, 0:P].rearrange("p (a w) -> p a w", a=1)
                            .broadcast_to([P, 2, P]))

                    def pv(k=k, ki=ki, t1=t1, pt=pt, ot_a=ot_a, ot_b=ot_b):
                        st_flag = (ki == 0)
                        sp_flag = (ki == nb - 1)
                        nc.tensor.matmul(ot_a[:, t1:TCH],
                                         vp_all[:, k, 2 * p, :],
                                         pt[:, 0, t1:TCH],
                                         start=st_flag, stop=sp_flag,
                                         skip_group_check=True)
                        nc.tensor.matmul(ot_b[:, t1:TCH],
                                         vp_all[:, k, 2 * p + 1, :],
                                         pt[:, 1, t1:TCH],
                                         start=st_flag, stop=sp_flag,
                                         skip_group_check=True)
                    pv_q.append(pv)
                    if len(pv_q) > PV_LAG:
                        pv_q.popleft()()
                    pace_fillers()
                while pv_q:
                    pv_q.popleft()()
                # let filler moves/copies go ahead of this pair's normalize on
                # the in-order DVE queue — they recycle PSUM banks that pending
                # PE matmuls wait on (drain everything before the last pair)
                for _ in range(6 if p < NPAIR - 1 else len(fillers)):
                    pop_filler()
                normalize(c, p, ot_a, ot_b)
            # drain remaining fillers before the next chunk's attention
            while fillers:
                fillers.popleft()()

        # ================= prologue =================
        # First-chunk inputs first so the PE can start ASAP; DMAs fire in
        # consumption order so the first matmul's two dependencies (xe
        # e0-slice + wq e0-slice, 128 KB each) sit at the queue head.  QK
        # weights load once here (resident for all chunks); wp defers into
        # window 0's fillers.
        xv0 = xT[0]
        xt0 = xe_pool.tile([P, ECH, TCH], BF16, name="xe0", tag="xe")
        xeg[0] = xt0
        dma(xt0[:, 0, :], xv0[:, 0, :])              # x e-chunk 0
        fill0 = deque(proj_closures(0))
        load_wqk(0, [0])                             # wq e0
        dma(xt0[:, 1, :], xv0[:, 1, :])              # x e-chunk 1
        load_wqk(0, [1])                             # wq e1
        dma(xt0[:, 2:4, :], xv0[:, 2:4, :])          # x e2,3
        load_wqk(0, [2, 3])                          # wq e2,3
        dma(xt0[:, 4:8, :], xv0[:, 4:8, :])          # x e4..7
        load_wqk(0, [4, 5, 6, 7])                    # wq e4..7
        load_wqk(1, [0, 1, 2, 3])                    # wk e0..3
        load_wqk(1, [4, 5, 6, 7])                    # wk e4..7
        dma(bqk_sb[:], bqk.rearrange("p (a b) -> p a b", a=2))
        dma(bv_sb[:], bvb[:, :])
        for g in range(4):
            dma(wv_sb[:, 2 * g:2 * g + 2, :], wv[:, 2 * g:2 * g + 2, :])
        dma(msk_sb[:], msk[:, :])
        nc.vector.memset(vp_all[:, :, :, D:D + 1], 1.0)
        for _ in range(6 * 9):  # q(p0), k(p0), V(i0..3) dense
            fill0.popleft()()

        def load_wp():
            dma(wp_sb[:], wp)
            dma(wp_hib[:], wp[D:P, NPAIR - 1])

        # ================= main windows =================
        # outproj(0) fills window 1; outproj(1)+(2) fill window 3, which has
        # no projection work left but the most attention (Act-bound) blocks
        op_sched = {1: (0,), 2: (), 3: (1, 2)}
        for c in range(NTCH):
            fillers = deque()
            guards = None
            if c == 0:
                fillers.extend(fill0)  # q/k chains for pairs 1..3
                fillers.append(load_wp)
                guards = {1: 18, 2: 36, 3: 54}
            for cc in op_sched.get(c, ()):
                fillers.extend(outproj_closures(cc))
            if c + 1 < NTCH:
                emit_xe_dma(c + 1)
                fillers.extend(proj_closures(c + 1))
            window(c, fillers, guards)

        # ============ final output projection (chunk 3) ============
        # Emitted so PE never head-of-line blocks on the last pair's
        # normalize: p0-p2 accumulations (ready) go first two chains at a
        # time; each chain's p3 is split into two K=64 matmuls (otn head-a +
        # un-shifted otb head-b); result copies ride the now-idle Act engine.
        cc = NTCH - 1
        chains = [(i, ec) for i in range(4) for ec in range(2)]
        yps = {}
        ysbs = {}
        # attention is over: the ot/st pools' PSUM banks are free and their
        # slot sizes fit a [P, TCH] f32 accumulator, so the final outproj can
        # run 6 chains in flight instead of 2
        # ot slots last: their previous occupants (the final pair's ot_a/b)
        # are only freed by the last normalize's reads
        yf_pools = [(ps512, "ps512"), (ps512, "ps512"), (st_pool, "st"),
                    (st_pool, "st"), (ot_pool, "ot"), (ot_pool, "ot")]

        def p012(n):
            i, ec = chains[n]
            pool, tag = yf_pools[n % len(yf_pools)]
            yp = pool.tile([P, TCH], F32, name=f"yf{i}_{ec}", tag=tag)
            yps[n] = yp
            for p in range(3):
                nc.tensor.matmul(yp[:], otn_all[(cc, p)][:, i * P:(i + 1) * P],
                                 wp_sb[:, p, ec, :], start=(p == 0),
                                 stop=False, skip_group_check=True)

        def p3_and_copy(n):
            i, ec = chains[n]
            yp = yps[n]
            otn3 = otn_all[(cc, NPAIR - 1)]
            otb3 = otb_last[0]
            nc.tensor.matmul(yp[:], otn3[0:D, i * P:(i + 1) * P],
                             wp_sb[0:D, NPAIR - 1, ec, :],
                             start=False, stop=False, skip_group_check=True)
            nc.tensor.matmul(yp[:], otb3[:, i * P:(i + 1) * P],
                             wp_hib[:, ec, :],
                             start=False, stop=True, skip_group_check=True)
            if i not in ysbs:
                ysbs[i] = ysb_pool.tile([P, 2, TCH], BF16, name=f"ysbf{i}",
                                        tag="ysb")
            if n % 2 == 0:
                nc.scalar.activation(ysbs[i][:, ec, :], yp[:], AF.Copy)
            else:
                nc.vector.tensor_copy(ysbs[i][:, ec, :], yp[:])
            if ec == 1:
                tt = 4 * cc + i
                dma(y[tt * P:(tt + 1) * P, :],
                    ysbs[i][:].rearrange("p a b -> p (a b)"))

        for n in range(6):
            p012(n)
        for n in range(len(chains)):
            p3_and_copy(n)
            if n + 6 < len(chains):
                p012(n + 6)


def make_io(nc):
    # x and all weights arrive host-pre-arranged in their exact SBUF layouts
    # so every DMA row is per-partition contiguous (8 KB rows — measured ~4x
    # better effective DMA throughput than the 1-2 KB rows a DRAM-side
    # rearrange produces)
    return {
        "xT": nc.dram_tensor("xT", [NTCH, P, ECH, TCH], BF16,
                             kind="ExternalInput").ap(),
        "wq": nc.dram_tensor("wq", [P, ECH, TCH], BF16,
                             kind="ExternalInput").ap(),
        "wk": nc.dram_tensor("wk", [P, ECH, TCH], BF16,
                             kind="ExternalInput").ap(),
        "wv": nc.dram_tensor("wv", [P, ECH, TCH], BF16,
                             kind="ExternalInput").ap(),
        "bqk": nc.dram_tensor("bqk", [P, 2 * NPAIR], F32,
                              kind="ExternalInput").ap(),
        "bvb": nc.dram_tensor("bvb", [P, H * D], F32, kind="ExternalInput").ap(),
        "wp": nc.dram_tensor("wp", [P, NPAIR, 2, TCH], BF16,
                             kind="ExternalInput").ap(),
        "msk": nc.dram_tensor("msk", [P, TCH], BF16, kind="ExternalInput").ap(),
        "y": nc.dram_tensor("y", [T, E], BF16, kind="ExternalOutput").ap(),
    }


def build(reps=1):
    """reps>1 emits the body multiple times into one NEFF (used by test.py's
    slope-based timing; kernel() always uses reps=1)."""
    nc = bacc.Bacc(trn_type="TRN2", target_bir_lowering=False, debug=False)
    io = make_io(nc)
    with tile.TileContext(nc) as tc:
        for _ in range(reps):
            _emit(nc, tc, io)
    nc.compile()
    return nc


def shard_inputs(inputs):
    """Full inputs -> per-core in_maps (8 cores: batch-major, group-minor)."""
    bx = np.asarray(inputs["batch_x"], np.float32)
    Wq = np.asarray(inputs["Wq"], np.float32)
    Wk = np.asarray(inputs["Wk"], np.float32)
    Wv = np.asarray(inputs["Wv"], np.float32)
    bq = np.asarray(inputs["bq"], np.float32)
    bk = np.asarray(inputs["bk"], np.float32)
    bv = np.asarray(inputs["bv"], np.float32)
    Wp = np.asarray(inputs["Wp"], np.float32)

    import ml_dtypes
    bf16 = ml_dtypes.bfloat16
    ps = np.arange(P, dtype=np.float32)[:, None]
    tf = np.arange(TCH, dtype=np.float32)[None, :]
    msk = (tf >= ps).astype(bf16)

    in_maps = []
    for core in range(N_CORES):
        b, g = core // 2, core % 2
        hs = slice(g * H, (g + 1) * H)
        bqf = bq[hs].reshape(NPAIR, P).T          # [128, 4] pair-major
        bkf = bk[hs].reshape(NPAIR, P).T

        def sb_w(W):
            # [E, H*D] -> SBUF layout [r=128, e-chunk=8, m=512]
            We = W[hs].transpose(1, 0, 2).reshape(E, H * D)
            return np.ascontiguousarray(
                We.reshape(ECH, P, H * D).transpose(1, 0, 2)).astype(bf16)

        xT = bx[b].T                              # [E, T]
        xh = np.ascontiguousarray(
            xT.reshape(ECH, P, NTCH, TCH).transpose(2, 1, 0, 3)).astype(bf16)
        Wpg = Wp[g * H * D:(g + 1) * H * D, :]    # [512, 1024]
        wph = np.ascontiguousarray(
            Wpg.reshape(NPAIR, P, 2, TCH).transpose(1, 0, 2, 3)).astype(bf16)
        in_maps.append({
            "xT": xh,
            "wq": sb_w(Wq),
            "wk": sb_w(Wk),
            "wv": sb_w(Wv),
            "bqk": np.ascontiguousarray(
                np.concatenate([bqf, bkf], axis=1)),
            "bvb": np.ascontiguousarray(
                np.tile(bv[hs].reshape(1, H * D), (P, 1))),
            "wp": wph,
            "msk": msk,
        })
    return in_maps


def gather_outputs(results, inputs):
    bp = np.asarray(inputs["bp"], np.float32)
    out = np.empty((B, T, E), np.float32)
    for b in range(B):
        out[b] = (np.asarray(results[2 * b]["y"], np.float32)
                  + np.asarray(results[2 * b + 1]["y"], np.float32)
                  + bp[None, :])
    return out


def _install_loud_hook():
    """Surface the real exception from the neuronx_cc PJRT callback."""
    import traceback
    from concourse import bass2jax
    try:
        import libneuronxla
    except ImportError:
        return
    orig = bass2jax.neuronx_cc_hook

    def loud(*a, **k):
        try:
            return orig(*a, **k)
        except BaseException:
            traceback.print_exc()
            raise

    if not hasattr(libneuronxla, "orig_neuronx_cc"):
        libneuronxla.orig_neuronx_cc = libneuronxla.neuronx_cc
    libneuronxla.neuronx_cc = loud
    bass2jax.install_neuronx_cc_hook = lambda: None


_NC_CACHE = []


def run(inputs, trace=False):
    _install_loud_hook()
    if not _NC_CACHE:
        _NC_CACHE.append(build())
    nc = _NC_CACHE[0]
    in_maps = shard_inputs(inputs)
    res = run_bass_kernel_spmd(nc, in_maps, core_ids=list(range(N_CORES)),
                               trace=trace)
    return gather_outputs(res.results, inputs), res


def kernel(**inputs):
    out, _ = run(inputs, trace=False)
    return out


# revision 26
# speedup vs baseline: 265.6234x; 1.0344x over previous
"""Multi-head causal attention on 8 Trainium2 NeuronCores.

Problem (hardcoded): batch_x [4, 2048, 1024], 16 heads x 64 head_size,
stacked per-head QKV params, causal softmax attention, output projection.

Sharding: 8 cores = 4 batches x 2 head-groups (8 heads each).  Each core
computes, for its (batch, head-group):
    QT/KT [hd=512, T] and V [T, hd=512] projections,
    ST = K @ Q^T per head (scores transposed: s on partitions, t free),
    P = exp(ST/8) with causal masking (upper s-blocks skipped entirely,
        diagonal blocks multiplied post-exp by a host-provided 0/1 mask),
    OT = V'^T @ P accumulated over s-blocks, where V' = [V | ones] so the
        softmax denominator accumulates in PSUM row 64 (M=65 matmuls),
    OT_norm = OT * (1/den)  (VectorE reciprocal + GPSIMD partition bcast),
    y_partial = OT_norm^T @ Wp_rows  (row-sharded output projection).
Host sums the two partials per batch and adds bp.

Dtypes: bf16 end-to-end on the PE (x, Wq/Wk/Wv, qt/kt, pt, vp, otn, Wp,
y); PSUM accumulation is always f32.  Measured on HW (unrolled-NEFF slope
method): a bf16 matmul with changing stationary weights costs ~157ns at
N=512 vs ~335ns for float32r (4-byte weight reload adds ~122ns/matmul;
bf16 reloads are free), so bf16 roughly halves real PE time for the
QKV-projection and score phases vs the f32r baseline.  Rel err ~4e-3 vs
the 2e-2 gate.

Layouts: x and every weight arrive host-pre-arranged in their exact SBUF
layouts so each DMA row is per-partition contiguous (8 KB rows; measured
~4x better effective DMA throughput than 1-2 KB rearrange rows).  Wq/Wk
are SBUF-resident (loaded once, not per chunk); bq/bk load as one [128,8]
tile instead of 512 4-byte descriptor rows.

Schedule: software-pipelined for PE occupancy (TimelineSim ~250us/core vs
~317us for the naive phase-sequential schedule):
  - exp() on the scalar engine is the per-block critical resource during
    attention (~170us busy with bf16 matmuls), so projection matmuls for
    chunk c+1 and delayed output projections are emitted as "filler" PE
    work between attention blocks (outproj(0) fills window 1;
    outproj(1)+(2) fill the Act-heaviest window 3; remainder drains at
    window end), and the QK projection moves run on DVE to stay off Act.
  - PV matmuls lag scores by PV_LAG blocks so exp/mask latency and the
    previous pair's normalize chain stay off the PE critical path; each
    pair runs its masked diagonal blocks first.
  - fillers are paced evenly over the window's remaining blocks (a lump
    draining densely at window end stalls the exp stream ~9-13us per
    boundary), and the QK projection moves alternate Act/DVE so the
    ps512 PSUM recycling never queues deep behind either engine; pt
    runs 12 deep (2 slack over PV_LAG's 10 in-flight) and otn 16 deep
    (all live tiles, no recycling waits) — paired A/B: -14us/body.
    (Measured dead ends, kept for the record: evacuating the ot PSUM
    banks to SBUF before the normalize chain costs more on the DVE
    queue than it saves; gpsimd partition_broadcast is ~2.5us/call on
    real HW but sits far enough off the critical path.)
  - fillers are drained ahead of each pair's normalize so PSUM-recycling
    moves aren't stuck behind it on the in-order DVE queue.
  - the final output projection runs 6 PSUM banks wide (reusing the done
    ot/st pool banks), splits its last-pair matmuls K=64 to read the
    un-shifted head-b tile, and alternates result copies Act/DVE.
  - first-chunk input DMAs (split small) precede the one-time weight
    loads so the PE starts ~5us in instead of ~26us.
"""

import numpy as np
from collections import deque
from contextlib import ExitStack

import concourse.bass as bass
import concourse.bacc as bacc
import concourse.mybir as mybir
import concourse.tile as tile
from concourse import library_config
from concourse.bass_utils import run_bass_kernel_spmd

# problem shape (hardcoded per contest rules)
B = 4
T = 2048
E = 1024
NH = 16          # total heads
D = 64           # head size
SCALE = 1.0 / 8.0  # 1/sqrt(64)

# per-core decomposition
H = 8            # heads per core
NPAIR = 4        # head pairs per core
TCH = 512        # t-chunk (matmul free dim)
NTCH = T // TCH  # 4
P = 128
ECH = E // P     # 8 e-chunks
NSB = T // P     # 16 s-blocks
N_CORES = 8
PV_LAG = 8       # blocks between score emission and PV emission

F32 = mybir.dt.float32
F32R = mybir.dt.float32r
BF16 = mybir.dt.bfloat16
AF = mybir.ActivationFunctionType
ALU = mybir.AluOpType


def _emit(nc, tc, io):
    xT, wq, wk, wv, bqk, bvb, wp, msk, y = (
        io["xT"], io["wq"], io["wk"], io["wv"], io["bqk"],
        io["bvb"], io["wp"], io["msk"], io["y"])

    ctx = ExitStack()
    with ctx:
        # ---- resident SBUF pools (bufs=1) ----
        res = ctx.enter_context(tc.tile_pool(name="res", bufs=1))
        kt_all = res.tile([P, NPAIR, T], BF16)          # KT: partitions = pair-hd
        vp_all = res.tile([P, NSB, H, D + 1], BF16)     # V' = [V | ones]
        wv_sb = res.tile([P, ECH, TCH], BF16)
        # resident QK weights, loaded once per e-chunk (all pairs wide)
        wq_sb = res.tile([P, ECH, NPAIR * P], BF16)
        wk_sb = res.tile([P, ECH, NPAIR * P], BF16)
        wp_sb = res.tile([P, NPAIR, 2, TCH], BF16)
        bqk_sb = res.tile([P, 2, NPAIR], F32)           # [:,0,:]=bq [:,1,:]=bk
        bv_sb = res.tile([P, TCH], F32)
        msk_sb = res.tile([P, TCH], BF16)
        # pair-3 head-b rows of Wp duplicated at partitions 0-63 for the
        # final outproj's K=64 split (otb lives on partitions 0-63)
        wp_hib = res.tile([D, 2, TCH], BF16)

        # ---- cycling pools ----
        xe_pool = ctx.enter_context(tc.tile_pool(name="xe", bufs=2))
        qt_pool = ctx.enter_context(tc.tile_pool(name="qt", bufs=2))
        pt_pool = ctx.enter_context(tc.tile_pool(name="pt", bufs=12))
        otn_pool = ctx.enter_context(tc.tile_pool(name="otn", bufs=16))
        rden_pool = ctx.enter_context(tc.tile_pool(name="rden", bufs=2))
        bc_pool = ctx.enter_context(tc.tile_pool(name="bc", bufs=1))
        otb_pool = ctx.enter_context(tc.tile_pool(name="otb", bufs=1))
        ysb_pool = ctx.enter_context(tc.tile_pool(name="ysb", bufs=4))
        ps512 = ctx.enter_context(tc.tile_pool(name="ps512", bufs=2, space="PSUM"))
        st_pool = ctx.enter_context(tc.tile_pool(name="stp", bufs=2, space="PSUM"))
        ot_pool = ctx.enter_context(tc.tile_pool(name="otp", bufs=2, space="PSUM"))

        dma = nc.sync.dma_start
        nc.gpsimd.load_library(library_config.attnmlp)

        xeg = {}   # c -> xe tile [P, ECH, TCH]
        qt_c = {}  # c -> qt tile [P, NPAIR, TCH]
        otn_all = {}  # (c, p) -> otn tile [P, TCH]
        otb_last = [None]  # un-shifted head-b tile of the very last pair

        def emit_xe_dma(c):
            xt = xe_pool.tile([P, ECH, TCH], BF16, name=f"xe{c}", tag="xe")
            dma(xt, xT[c])
            xeg[c] = xt

        def load_wqk(proj, es):
            # one [P, |es|, 512] load covers e-chunks `es` for all 4 pairs;
            # host pre-arranged to SBUF layout, so rows are contiguous
            wsb = (wq_sb, wk_sb)[proj]
            src = (wq, wk)[proj]
            dma(wsb[:, es[0]:es[-1] + 1, :], src[:, es[0]:es[-1] + 1, :])

        def proj_closures(c):
            """Closure list for chunk c's QKV projections, in emission order
            [q(p0), k(p0), V(i0..3), q(p1), k(p1), ..., q(p3), k(p3)] — the
            prologue drains the first 6 chains dense, window fillers take the
            rest.  Each chain is 8 matmuls + a move w/ bias.  QK weights are
            resident (wq_sb/wk_sb, loaded once in the prologue)."""
            t0 = c * TCH

            def xe_rhs(e):
                return xeg[c][:, e, :]

            def xe_lhs(e, i):
                return xeg[c][:, e, i * P:(i + 1) * P]

            qt_c[c] = qt_pool.tile([P, NPAIR, TCH], BF16, name=f"qtc{c}",
                                   tag="qt")
            chains = []  # (proj, p, dest, label) per q/k chain
            for p in range(NPAIR):
                chains.append((0, p, qt_c[c][:, p, :], f"q{c}_{p}"))
                chains.append((1, p, kt_all[:, p, t0:t0 + TCH],
                               f"k{c}_{p}"))

            def qk_ops(n):
                ops = []
                proj, p, dest, label = chains[n]
                wsb = (wq_sb, wk_sb)[proj]
                ps = ps512.tile([P, TCH], F32, name=f"ps{label}", tag="ps512")
                for e in range(ECH):
                    def mm(proj=proj, p=p, ps=ps, e=e, wsb=wsb):
                        nc.tensor.matmul(ps[:], wsb[:, e, p * P:(p + 1) * P],
                                         xe_rhs(e),
                                         start=(e == 0), stop=(e == ECH - 1),
                                         skip_group_check=True)
                    ops.append(mm)

                def mv(n=n, proj=proj, p=p, dest=dest, ps=ps):
                    # alternate Act/DVE: these moves recycle the ps512 PSUM
                    # banks the next filler chain waits on, so queue latency
                    # on either single engine stalls the PE (model: ~1-2us
                    # per chain); splitting halves the in-order queue depth
                    if n % 2:
                        nc.scalar.activation(dest, ps[:], AF.Identity,
                                             bias=bqk_sb[:, proj, p:p + 1])
                    else:
                        nc.vector.tensor_add(
                            dest, ps[:],
                            bqk_sb[:, proj, p:p + 1].broadcast_to([P, TCH]))
                ops.append(mv)
                return ops

            def v_ops(i):
                ops = []
                tt = 4 * c + i
                ps = ps512.tile([P, TCH], F32, name=f"v{c}_{i}", tag="ps512")
                for e in range(ECH):
                    def mm(ps=ps, e=e, i=i):
                        nc.tensor.matmul(ps[:], xe_lhs(e, i), wv_sb[:, e, :],
                                         start=(e == 0), stop=(e == ECH - 1),
                                         skip_group_check=True)
                    ops.append(mm)

                def mv(ps=ps, tt=tt):
                    nc.vector.tensor_add(
                        vp_all[:, tt, :, 0:D],
                        ps[:].rearrange("p (h d) -> p h d", d=D),
                        bv_sb[:].rearrange("p (h d) -> p h d", d=D))
                ops.append(mv)
                return ops

            ops = qk_ops(0) + qk_ops(1)
            for i in range(4):
                ops += v_ops(i)
            for n in range(2, len(chains)):
                ops += qk_ops(n)
            return ops

        def outproj_closures(cc):
            """Filler closures for the output projection of chunk cc."""
            ops = []
            for i in range(4):
                tt = 4 * cc + i
                ysb = ysb_pool.tile([P, 2, TCH], BF16, name=f"ysb{cc}_{i}",
                                    tag="ysb")
                for ec in range(2):
                    yp = ps512.tile([P, TCH], F32, name=f"y{cc}_{i}_{ec}",
                                    tag="ps512")
                    for p in range(NPAIR):
                        def mm(cc=cc, i=i, ec=ec, p=p, yp=yp):
                            nc.tensor.matmul(
                                yp[:], otn_all[(cc, p)][:, i * P:(i + 1) * P],
                                wp_sb[:, p, ec, :],
                                start=(p == 0), stop=(p == NPAIR - 1),
                                skip_group_check=True)
                        ops.append(mm)

                    def cp(ysb=ysb, ec=ec, yp=yp):
                        nc.vector.tensor_copy(ysb[:, ec, :], yp[:])
                    ops.append(cp)

                def st(tt=tt, ysb=ysb):
                    dma(y[tt * P:(tt + 1) * P, :],
                        ysb[:].rearrange("p a b -> p (a b)"))
                ops.append(st)
            return ops

        def normalize(c, p, ot_a, ot_b):
            rden_a = rden_pool.tile([1, TCH], F32, name=f"rda{c}_{p}", tag="rd")
            rden_b = rden_pool.tile([1, TCH], F32, name=f"rdb{c}_{p}", tag="rd")
            nc.vector.reciprocal(rden_a[:], ot_a[D:D + 1, :])
            nc.vector.reciprocal(rden_b[:], ot_b[D:D + 1, :])
            bc_a = bc_pool.tile([D, TCH], F32, name=f"bca{c}_{p}", tag="bca")
            bc_b = bc_pool.tile([D, TCH], F32, name=f"bcb{c}_{p}", tag="bcb")
            nc.gpsimd.partition_broadcast(bc_a[:], rden_a[:], channels=D)
            nc.gpsimd.partition_broadcast(bc_b[:], rden_b[:], channels=D)
            otn = otn_pool.tile([P, TCH], BF16, name=f"otn{c}_{p}", tag="otn")
            otb = otb_pool.tile([D, TCH], BF16, name=f"otb{c}_{p}", tag="otb")
            nc.vector.tensor_mul(otn[0:D, :], ot_a[0:D, :], bc_a[:])
            nc.vector.tensor_mul(otb[:], ot_b[0:D, :], bc_b[:])
            if (c, p) == (NTCH - 1, NPAIR - 1):
                # last pair of the kernel: the final outproj reads otb
                # directly via a K=64 matmul split, so the ~2us DMA shift
                # latency stays off the critical tail
                otb_last[0] = otb
            else:
                # partition shift 0:64 -> 64:128 (DMA; DVE lanes can't shift)
                dma(otn[D:2 * D, :], otb[:])
            otn_all[(c, p)] = otn

        def window(c, fillers, guards=None):
            """Attention for chunk c, interleaved with filler PE work.

            guards[p] = minimum number of fillers that must be EMITTED before
            pair p's first score (Tile versioning is emission-ordered, so a
            score reading qt/kt written by a not-yet-emitted filler move would
            silently read the previous version)."""
            nb = 4 * (c + 1)  # causal s-blocks for this chunk
            popped = [0]
            blocks_done = [0]
            total_blocks = NPAIR * nb

            def pop_filler():
                if fillers:
                    fillers.popleft()()
                    popped[0] += 1

            def pace_fillers():
                # spread remaining fillers evenly over remaining blocks: a
                # leftover lump draining densely at window end stalls the
                # exp stream ~9-13us per boundary (next window's scores only
                # emit after the drain)
                blocks_done[0] += 1
                rem = total_blocks - blocks_done[0]
                if rem <= 0:
                    return
                per = -(-len(fillers) // rem)   # ceil
                for _ in range(per):
                    pop_filler()

            for p in range(NPAIR):
                if guards:
                    while popped[0] < guards.get(p, 0) and fillers:
                        pop_filler()
                ot_a = ot_pool.tile([D + 1, TCH], F32, name=f"ota{c}_{p}",
                                    tag="ot")
                ot_b = ot_pool.tile([D + 1, TCH], F32, name=f"otb{c}_{p}",
                                    tag="ot")
                pv_q = deque()
                # diagonal blocks first: their mask multiplies land on the
                # in-order DVE queue while it still has the previous pair's
                # normalize queued, and the PV lag covers both; the unmasked
                # wide blocks then flush the pair densely.  Pair 0 keeps them
                # last — at window start DVE is busy with boundary moves.
                ks = list(range(4 * c, nb)) + list(range(0, 4 * c))
                for ki, k in enumerate(ks):
                    # diagonal blocks: columns < 128j are fully masked; trim
                    # all work to the live range [t1:TCH] (t1 = 128j).
                    j = k - 4 * c
                    t1 = 128 * j if j > 0 else 0
                    w = TCH - t1
                    st = st_pool.tile([P, 2, TCH], F32, name=f"st{c}_{p}_{k}",
                                      tag="st")
                    pt = pt_pool.tile([P, 2, TCH], BF16, name=f"pt{c}_{p}_{k}",
                                      tag="pt")
                    for h in (0, 1):
                        lo = 64 * h
                        nc.tensor.matmul(
                            st[:, h, t1:TCH],
                            kt_all[lo:lo + 64, p, k * P:(k + 1) * P],
                            qt_c[c][lo:lo + 64, p, t1:TCH],
                            start=True, stop=True)
                    nc.scalar.activation(pt[:, :, t1:TCH], st[:, :, t1:TCH],
                                         AF.Exp, scale=SCALE)
                    if j >= 0:
                        # the mask is only != 1 inside the 128-wide diagonal
                        # square [t1, t1+128): beyond it t >= 128j+s for all
                        # s, so the multiply needn't touch those columns
                        nc.vector.tensor_mul(
                            pt[:, :, t1:t1 + P], pt[:, :, t1:t1 + P],
                            msk_sb[:, 0:P].rearrange("p (a w) -> p a w", a=1)
                            .broadcast_to([P, 2, P]))

                    def pv(k=k, ki=ki, t1=t1, pt=pt, ot_a=ot_a, ot_b=ot_b):
                        st_flag = (ki == 0)
                        sp_flag = (ki == nb - 1)
                        nc.tensor.matmul(ot_a[:, t1:TCH],
                                         vp_all[:, k, 2 * p, :],
                                         pt[:, 0, t1:TCH],
                                         start=st_flag, stop=sp_flag,
                                         skip_group_check=True)
                        nc.tensor.matmul(ot_b[:, t1:TCH],
                                         vp_all[:, k, 2 * p + 1, :],
                                         pt[:, 1, t1:TCH],
                                         start=st_flag, stop=sp_flag,
                                         skip_group_check=True)
                    pv_q.append(pv)
                    if len(pv_q) > PV_LAG:
                        pv_q.popleft()()
                    pace_fillers()
                while pv_q:
                    pv_q.popleft()()
                # let filler moves/copies go ahead of this pair's normalize on
                # the in-order DVE queue — they recycle PSUM banks that pending
                # PE matmuls wait on (drain everything before the last pair)
                for _ in range(6 if p < NPAIR - 1 else len(fillers)):
                    pop_filler()
                normalize(c, p, ot_a, ot_b)
            # drain remaining fillers before the next chunk's attention
            while fillers:
                fillers.popleft()()

        # ================= prologue =================
        # First-chunk inputs first so the PE can start ASAP; DMAs fire in
        # consumption order so the first matmul's two dependencies (xe
        # e0-slice + wq e0-slice, 128 KB each) sit at the queue head.  QK
        # weights load once here (resident for all chunks); wp defers into
        # window 0's fillers.
        xv0 = xT[0]
        xt0 = xe_pool.tile([P, ECH, TCH], BF16, name="xe0", tag="xe")
        xeg[0] = xt0
        dma(xt0[:, 0, :], xv0[:, 0, :])              # x e-chunk 0
        fill0 = deque(proj_closures(0))
        load_wqk(0, [0])                             # wq e0
        dma(xt0[:, 1, :], xv0[:, 1, :])              # x e-chunk 1
        load_wqk(0, [1])                             # wq e1
        dma(xt0[:, 2:4, :], xv0[:, 2:4, :])          # x e2,3
        load_wqk(0, [2, 3])                          # wq e2,3
        dma(xt0[:, 4:8, :], xv0[:, 4:8, :])          # x e4..7
        load_wqk(0, [4, 5, 6, 7])                    # wq e4..7
        load_wqk(1, [0, 1, 2, 3])                    # wk e0..3
        load_wqk(1, [4, 5, 6, 7])                    # wk e4..7
        dma(bqk_sb[:], bqk.rearrange("p (a b) -> p a b", a=2))
        dma(bv_sb[:], bvb[:, :])
        for g in range(4):
            dma(wv_sb[:, 2 * g:2 * g + 2, :], wv[:, 2 * g:2 * g + 2, :])
        dma(msk_sb[:], msk[:, :])
        nc.vector.memset(vp_all[:, :, :, D:D + 1], 1.0)
        for _ in range(6 * 9):  # q(p0), k(p0), V(i0..3) dense
            fill0.popleft()()

        def load_wp():
            dma(wp_sb[:], wp)
            dma(wp_hib[:], wp[D:P, NPAIR - 1])

        # ================= main windows =================
        # outproj(0) fills window 1; outproj(1)+(2) fill window 3, which has
        # no projection work left but the most attention (Act-bound) blocks
        op_sched = {1: (0,), 2: (), 3: (1, 2)}
        for c in range(NTCH):
            fillers = deque()
            guards = None
            if c == 0:
                fillers.extend(fill0)  # q/k chains for pairs 1..3
                fillers.append(load_wp)
                guards = {1: 18, 2: 36, 3: 54}
            for cc in op_sched.get(c, ()):
                fillers.extend(outproj_closures(cc))
            if c + 1 < NTCH:
                emit_xe_dma(c + 1)
                fillers.extend(proj_closures(c + 1))
            window(c, fillers, guards)

        # ============ final output projection (chunk 3) ============
        # Emitted so PE never head-of-line blocks on the last pair's
        # normalize: p0-p2 accumulations (ready) go first two chains at a
        # time; each chain's p3 is split into two K=64 matmuls (otn head-a +
        # un-shifted otb head-b); result copies ride the now-idle Act engine.
        cc = NTCH - 1
        chains = [(i, ec) for i in range(4) for ec in range(2)]
        yps = {}
        ysbs = {}
        # attention is over: the ot/st pools' PSUM banks are free and their
        # slot sizes fit a [P, TCH] f32 accumulator, so the final outproj can
        # run 6 chains in flight instead of 2
        # ot slots last: their previous occupants (the final pair's ot_a/b)
        # are only freed by the last normalize's reads
        yf_pools = [(ps512, "ps512"), (ps512, "ps512"), (st_pool, "st"),
                    (st_pool, "st"), (ot_pool, "ot"), (ot_pool, "ot")]

        def p012(n):
            i, ec = chains[n]
            pool, tag = yf_pools[n % len(yf_pools)]
            yp = pool.tile([P, TCH], F32, name=f"yf{i}_{ec}", tag=tag)
            yps[n] = yp
            for p in range(3):
                nc.tensor.matmul(yp[:], otn_all[(cc, p)][:, i * P:(i + 1) * P],
                                 wp_sb[:, p, ec, :], start=(p == 0),
                                 stop=False, skip_group_check=True)

        def p3_and_copy(n):
            i, ec = chains[n]
            yp = yps[n]
            otn3 = otn_all[(cc, NPAIR - 1)]
            otb3 = otb_last[0]
            nc.tensor.matmul(yp[:], otn3[0:D, i * P:(i + 1) * P],
                             wp_sb[0:D, NPAIR - 1, ec, :],
                             start=False, stop=False, skip_group_check=True)
            nc.tensor.matmul(yp[:], otb3[:, i * P:(i + 1) * P],
                             wp_hib[:, ec, :],
                             start=False, stop=True, skip_group_check=True)
            if i not in ysbs:
                ysbs[i] = ysb_pool.tile([P, 2, TCH], BF16, name=f"ysbf{i}",
                                        tag="ysb")
            if n % 2 == 0:
                nc.scalar.activation(ysbs[i][:, ec, :], yp[:], AF.Copy)
            else:
                nc.vector.tensor_copy(ysbs[i][:, ec, :], yp[:])
            if ec == 1:
                tt = 4 * cc + i
                dma(y[tt * P:(tt + 1) * P, :],
                    ysbs[i][:].rearrange("p a b -> p (a b)"))

        for n in range(6):
            p012(n)
        for n in range(len(chains)):
            p3_and_copy(n)
            if n + 6 < len(chains):
                p012(n + 6)


def make_io(nc):
    # x and all weights arrive host-pre-arranged in their exact SBUF layouts
    # so every DMA row is per-partition contiguous (8 KB rows — measured ~4x
    # better effective DMA throughput than the 1-2 KB rows a DRAM-side
    # rearrange produces)
    return {
        "xT": nc.dram_tensor("xT", [NTCH, P, ECH, TCH], BF16,
                             kind="ExternalInput").ap(),
        "wq": nc.dram_tensor("wq", [P, ECH, TCH], BF16,
                             kind="ExternalInput").ap(),
        "wk": nc.dram_tensor("wk", [P, ECH, TCH], BF16,
                             kind="ExternalInput").ap(),
        "wv": nc.dram_tensor("wv", [P, ECH, TCH], BF16,
                             kind="ExternalInput").ap(),
        "bqk": nc.dram_tensor("bqk", [P, 2 * NPAIR], F32,
                              kind="ExternalInput").ap(),
        "bvb": nc.dram_tensor("bvb", [P, H * D], F32, kind="ExternalInput").ap(),
        "wp": nc.dram_tensor("wp", [P, NPAIR, 2, TCH], BF16,
                             kind="ExternalInput").ap(),
        "msk": nc.dram_tensor("msk", [P, TCH], BF16, kind="ExternalInput").ap(),
        "y": nc.dram_tensor("y", [T, E], BF16, kind="ExternalOutput").ap(),
    }


def build(reps=1):
    """reps>1 emits the body multiple times into one NEFF (used by test.py's
    slope-based timing; kernel() always uses reps=1)."""
    nc = bacc.Bacc(trn_type="TRN2", target_bir_lowering=False, debug=False)
    io = make_io(nc)
    with tile.TileContext(nc) as tc:
        for _ in range(reps):
            _emit(nc, tc, io)
    nc.compile()
    return nc


def shard_inputs(inputs):
    """Full inputs -> per-core in_maps (8 cores: batch-major, group-minor)."""
    bx = np.asarray(inputs["batch_x"], np.float32)
    Wq = np.asarray(inputs["Wq"], np.float32)
    Wk = np.asarray(inputs["Wk"], np.float32)
    Wv = np.asarray(inputs["Wv"], np.float32)
    bq = np.asarray(inputs["bq"], np.float32)
    bk = np.asarray(inputs["bk"], np.float32)
    bv = np.asarray(inputs["bv"], np.float32)
    Wp = np.asarray(inputs["Wp"], np.float32)

    import ml_dtypes
    bf16 = ml_dtypes.bfloat16
    ps = np.arange(P, dtype=np.float32)[:, None]
    tf = np.arange(TCH, dtype=np.float32)[None, :]
    msk = (tf >= ps).astype(bf16)

    in_maps = []
    for core in range(N_CORES):
        b, g = core // 2, core % 2
        hs = slice(g * H, (g + 1) * H)
        bqf = bq[hs].reshape(NPAIR, P).T          # [128, 4] pair-major
        bkf = bk[hs].reshape(NPAIR, P).T

        def sb_w(W):
            # [E, H*D] -> SBUF layout [r=128, e-chunk=8, m=512]
            We = W[hs].transpose(1, 0, 2).reshape(E, H * D)
            return np.ascontiguousarray(
                We.reshape(ECH, P, H * D).transpose(1, 0, 2)).astype(bf16)

        xT = bx[b].T                              # [E, T]
        xh = np.ascontiguousarray(
            xT.reshape(ECH, P, NTCH, TCH).transpose(2, 1, 0, 3)).astype(bf16)
        Wpg = Wp[g * H * D:(g + 1) * H * D, :]    # [512, 1024]
        wph = np.ascontiguousarray(
            Wpg.reshape(NPAIR, P, 2, TCH).transpose(1, 0, 2, 3)).astype(bf16)
        in_maps.append({
            "xT": xh,
            "wq": sb_w(Wq),
            "wk": sb_w(Wk),
            "wv": sb_w(Wv),
            "bqk": np.ascontiguousarray(
                np.concatenate([bqf, bkf], axis=1)),
            "bvb": np.ascontiguousarray(
                np.tile(bv[hs].reshape(1, H * D), (P, 1))),
            "wp": wph,
            "msk": msk,
        })
    return in_maps


def gather_outputs(results, inputs):
    bp = np.asarray(inputs["bp"], np.float32)
    out = np.empty((B, T, E), np.float32)
    for b in range(B):
        out[b] = (np.asarray(results[2 * b]["y"], np.float32)
                  + np.asarray(results[2 * b + 1]["y"], np.float32)
                  + bp[None, :])
    return out


def _install_loud_hook():
    """Surface the real exception from the neuronx_cc PJRT callback."""
    import traceback
    from concourse import bass2jax
    try:
        import libneuronxla
    except ImportError:
        return
    orig = bass2jax.neuronx_cc_hook

    def loud(*a, **k):
        try:
            return orig(*a, **k)
        except BaseException:
            traceback.print_exc()
            raise

    if not hasattr(libneuronxla, "orig_neuronx_cc"):
        libneuronxla.orig_neuronx_cc = libneuronxla.neuronx_cc
    libneuronxla.neuronx_cc = loud
    bass2jax.install_neuronx_cc_hook = lambda: None


_NC_CACHE = []


def run(inputs, trace=False):
    _install_loud_hook()
    if not _NC_CACHE:
        _NC_CACHE.append(build())
    nc = _NC_CACHE[0]
    in_maps = shard_inputs(inputs)
    res = run_bass_kernel_spmd(nc, in_maps, core_ids=list(range(N_CORES)),
                               trace=trace)
    return gather_outputs(res.results, inputs), res


def kernel(**inputs):
    out, _ = run(inputs, trace=False)
    return out
